# revision 1
# baseline (speedup 1.0000x reference)
"""Trainium2 Bass kernel for nn_MetaQDA_FB (MetaQDA Fisher-Bayes logits).

Math: sigma_c = scale * (L L^T + V_c V_c^T) with V_c = [Xc_c^T, sqrt(beta)(mean_c-m)]
rank-17 (padded to 18), so per-class inversion/logdet reduces to a shared
triangular inverse W = L^{-1} (blocked Neumann + forward substitution on PE)
plus 18x18 capacitance matrices M_c = I + (W V_c)^T (W V_c), inverted in a
batched Gauss-Jordan sweep on the vector engine (one class per partition).
Queries are sharded across the 8 cores (256 each); every core redundantly
builds the (cheap) per-class data and scores its own query block:

  maha_qc = (1-REG)/scale * (||W(x_q-mu_c)||^2 - g^T K_c g) + REG ||x_q-mu_c||^2
  logits  = bias_c - 0.5(common+d) * log1p(maha/common)

Everything O(n^3) runs on device; the host only reorders/transposes inputs,
computes O(1) scalars, and gathers the per-core [C, 256] logit shards.
"""

import math
import sys

import numpy as np

for _p in ("/opt/trn_rl_repo",):
    if _p not in sys.path:
        sys.path.append(_p)

D, C, S, Q, REG, EPS = 640, 64, 16, 2048, 0.3, 1e-6
B = 128
NB = D // B            # 5 row/col blocks of L
R = 18                 # padded low-rank stride (S + 1 -> 18)
GC = 7                 # classes per 126-partition group
NG = (C + GC - 1) // GC
NCORES = 8
QS = Q // NCORES       # queries per core
NEUMANN = 9            # Neumann order for the diagonal block inverses
STOP_AFTER = 99        # debug: truncate kernel after phase N
F32R = False           # use float32r (1 cyc/row at N>=256) for the scoring matmuls
F32 = np.float32


def _host_prep(inputs):
    Xs = np.asarray(inputs["X_support"], dtype=F32)
    y = np.asarray(inputs["y"])
    Xq = np.asarray(inputs["X_query"], dtype=F32)
    m = np.asarray(inputs["m"], dtype=F32).reshape(-1)
    kappa = float(np.asarray(inputs["kappa"]))
    nu = float(np.asarray(inputs["nu"]))
    td = np.asarray(inputs["triu_diag"], dtype=F32).reshape(-1)
    tl = np.asarray(inputs["triu_lower"], dtype=F32)

    perm = np.argsort(y, kind="stable")
    XgT = np.ascontiguousarray(Xs[perm].T)                    # [D, C*S]

    mask = np.tril(np.ones((D, D), dtype=F32), k=-1)
    L = (np.diag(np.abs(td)) + tl * mask).astype(F32)
    LT = np.ascontiguousarray(L.T)                            # [D, D]
    LTdiagS = np.zeros((D, B), dtype=F32)                     # strict-upper diag blocks of LT
    for b in range(NB):
        blk = LT[b * B:(b + 1) * B, b * B:(b + 1) * B].copy()
        blk[np.tril_indices(B)] = 0.0
        LTdiagS[b * B:(b + 1) * B] = -blk    # negated: Neumann add of I happens in PSUM

    kappa_ = abs(kappa) + EPS
    nu_ = max(nu, D - 1 + EPS)
    Nj = float(S)
    scale = (kappa_ + Nj + 1.0) / ((nu_ + Nj - D + 1.0) * (kappa_ + Nj))
    common = nu_ + Nj + 1.0 - D
    beta = kappa_ * Nj / (kappa_ + Nj)
    BC0 = (math.lgamma(0.5 * (common + D)) - math.lgamma(0.5 * common)
           - 0.5 * D * math.log(common)
           - 0.5 * D * math.log(scale)
           + 0.5 * (common + D) * math.log(common))
    sc = dict(
        scale=scale, common=common, beta=beta,
        cmu1=kappa_ / (kappa_ + Nj), cmu2=Nj / (kappa_ + Nj),
        sbeta=math.sqrt(beta), ca=(1.0 - REG) / scale,
        BC0=BC0, CC=0.5 * (common + D), inv_s=1.0 / Nj,
    )

    ident = np.eye(B, dtype=F32)
    onesr = np.ones((B, C), dtype=F32)
    eyec = np.eye(C, dtype=F32)
    eyeflat = np.tile(np.eye(R, dtype=F32).reshape(1, R * R), (C, 1)).astype(F32)
    maskb = np.zeros((B, NG * C), dtype=F32)
    maski = np.zeros((B, NG * GC), dtype=F32)
    for g in range(NG):
        for lc in range(min(GC, C - g * GC)):
            maskb[lc * R:(lc + 1) * R, g * C + g * GC + lc] = 1.0
            maski[lc * R:(lc + 1) * R, g * GC + lc] = 1.0

    shared = dict(
        xgt=XgT, ltf=LT, ltds=LTdiagS,
        mcol=m.reshape(D, 1), tdcol=td.reshape(D, 1),
        ident=ident, onesr=onesr, eyec=eyec, eyeflat=eyeflat,
        maskb=maskb, maski=maski,
    )
    xqts = [np.ascontiguousarray(Xq[c * QS:(c + 1) * QS].T) for c in range(NCORES)]
    return shared, xqts, sc


def _emit(nc, tc, ins, sc):
    """Emit the whole kernel under an open TileContext."""
    import concourse.mybir as mybir
    from concourse.bass import ds

    fp = mybir.dt.float32
    AF = mybir.ActivationFunctionType
    OP = mybir.AluOpType
    AX = mybir.AxisListType

    pool = tc.alloc_tile_pool(name="persist", bufs=1)
    spool = tc.alloc_tile_pool(name="scratch", bufs=2)
    ps = tc.alloc_tile_pool(name="ps", bufs=7, space="PSUM")
    ps2 = tc.alloc_tile_pool(name="ps2", bufs=1, space="PSUM")

    def psum(shape, tag="ps"):
        return ps.tile(shape, fp, name=tag, tag="ps")

    def psum2(shape):
        return ps2.tile(shape, fp, name="pss", tag="pss")

    fpr = mybir.dt.float32r

    def mmr(out, lhsT, rhs, **kw):
        """matmul in float32r when enabled (full-rate fp32 streaming)."""
        if F32R:
            lhsT = lhsT.bitcast(fpr)
            rhs = rhs.bitcast(fpr)
        nc.tensor.matmul(out, lhsT, rhs, **kw)

    dma = nc.sync.dma_start

    _dma_engines = [nc.sync, nc.gpsimd, nc.scalar]
    _dma_rr = [0]

    def dma_small(out, in_):
        eng = _dma_engines[_dma_rr[0] % len(_dma_engines)]
        _dma_rr[0] += 1
        eng.dma_start(out, in_)

    # ---- persistent SBUF tensors ----
    T = lambda name, shape: pool.tile(shape, fp, name=name, tag=name)
    xg_sb = T("xg_sb", [B, NB * C * S])
    ltf_sb = T("ltf_sb", [B, NB * D])        # LT block (k,i) at [:, k*D + i*B]
    ltds_sb = T("ltds_sb", [B, NB * B])
    vbuf = T("vbuf", [B, NB * C * R])
    wsb = T("wsb", [B, NB * D])              # W block (i,j) at [:, i*D + j*B]
    wtsb = T("wtsb", [B, NB * D])            # W^T block (a,b) at [:, a*D + b*B]
    pbuf = T("pbuf", [B, NB * C * R])        # P block-i at [:, i*C*R ...]
    xmu_rhs = T("xmu_rhs", [B, NB * (QS + C)])   # [xq | mu] per k block
    tu_rhs = T("tu_rhs", [B, NB * (QS + C)])     # [t | u] per i block
    t2x2 = T("t2x2", [B, NB * 2 * QS])
    mean_sb = T("mean_sb", [B, NB * C])
    m_sb = T("m_sb", [B, NB])
    td_sb = T("td_sb", [B, NB])
    gbuf = T("gbuf", [B, NG * QS])
    mbuf = T("mbuf", [C, R * R])
    kdfull = T("kdfull", [B, NG * GC * R])
    corrbuf = T("corrbuf", [C, QS])
    tu_sb = T("tu_sb", [C, QS + C])
    xmu_sb = T("xmu_sb", [C, QS + C])
    tnxn_sb = T("tnxn_sb", [C, 2 * QS])
    logpiv = T("logpiv", [C, R])
    un_sb = T("un_sb", [C, 1])
    mun_sb = T("mun_sb", [C, 1])
    ld2_sb = T("ld2_sb", [C, 1])
    lda_sb = T("lda_sb", [C, 1])
    bias_sb = T("bias_sb", [C, 1])
    logits_sb = T("logits_sb", [C, QS])
    scr64 = T("scr64", [C, C])
    # consts
    ident = T("ident", [B, B])
    onesr = T("onesr", [B, C])
    eyec = T("eyec", [C, C])
    eyeflat = T("eyeflat", [C, R * R])
    maskb = T("maskb", [B, NG * C])
    maski = T("maski", [B, NG * GC])

    # ---- input DMAs (W-phase inputs first; spread queues) ----
    dma(ident[:], ins["ident"][:])
    dma(ltds_sb.rearrange("p (b n) -> p b n", b=NB),
        ins["ltds"].rearrange("(b p) n -> p b n", p=B))
    dma(ltf_sb.rearrange("p (b n) -> p b n", b=NB),
        ins["ltf"].rearrange("(b p) n -> p b n", p=B))
    nc.gpsimd.dma_start(xg_sb.rearrange("p (b n) -> p b n", b=NB),
                        ins["xgt"].rearrange("(b p) n -> p b n", p=B))
    nc.gpsimd.dma_start(xmu_rhs.rearrange("p (b n) -> p b n", n=QS + C)[:, :, 0:QS],
                        ins["xqt"].rearrange("(b p) n -> p b n", p=B))
    dma(m_sb[:], ins["mcol"].rearrange("(b p) one -> p (b one)", p=B))
    dma(td_sb[:], ins["tdcol"].rearrange("(b p) one -> p (b one)", p=B))
    for cname, ct in (("onesr", onesr), ("eyec", eyec),
                      ("eyeflat", eyeflat), ("maskb", maskb), ("maski", maski)):
        dma(ct[:], ins[cname][:])

    nc.vector.memset(kdfull[:], 0.0)

    def _gate(n):
        if STOP_AFTER <= n:
            nc.vector.memset(logits_sb[:], 0.0)
            dma(ins["out"][:], logits_sb[:])
            for p in (ps2, ps, spool, pool):
                p.release()
            return True
        return False

    if _gate(1):
        return

    lt_blk = lambda k, i: ltf_sb[:, k * D + i * B: k * D + (i + 1) * B]
    w_blk = lambda i, j: wsb[:, i * D + j * B: i * D + (j + 1) * B]
    wt_blk = lambda a, b: wtsb[:, a * D + b * B: a * D + (b + 1) * B]

    # =========== phase W: W = inv(L), blockwise ===========
    # iteration-major so all 5 chains pipeline through the shared PSUM slots
    s_prevs = [ident] * NB
    for it in range(NEUMANN):
        for b in range(NB):
            ecol = ltds_sb[:, b * B:(b + 1) * B]   # = -(E_bb)^T
            pm = psum([B, B])
            nc.tensor.matmul(pm[:], ecol, s_prevs[b], start=True, stop=False)
            nc.tensor.matmul(pm[:], ident[:], ident[:], start=False, stop=True)
            if it < NEUMANN - 1:
                s_new = spool.tile([B, B], fp, name=f"wS{b}_{it}", tag=f"wS{b}", bufs=3)
            else:
                s_new = w_blk(b, b)
            if b % 2 == 0:                         # S <- I - E S, built in PSUM
                nc.scalar.copy(s_new, pm[:])
            else:
                nc.vector.tensor_copy(s_new, pm[:])
            s_prevs[b] = s_new
    for b in range(NB):
        ptr = psum([B, B])
        nc.tensor.transpose(ptr[:], w_blk(b, b), ident[:])
        nc.scalar.copy(wt_blk(b, b), ptr[:])

    for d in range(1, NB):
      for j in range(NB - d):
        if True:
            i = j + d
            pacc = psum([B, B])
            for k in range(j, i):
                nc.tensor.matmul(pacc[:], lt_blk(k, i), w_blk(k, j),
                                 start=(k == j), stop=(k == i - 1))
            tij = spool.tile([B, B], fp, name=f"tij{i}{j}", tag="tij")
            nc.scalar.copy(tij[:], pacc[:])
            pw = psum([B, B])
            nc.tensor.matmul(pw[:], wt_blk(i, i), tij[:], start=True, stop=True)
            nc.vector.tensor_scalar(out=w_blk(i, j), in0=pw[:], scalar1=-1.0,
                                    scalar2=None, op0=OP.mult)
            ptr = psum([B, B])
            nc.tensor.transpose(ptr[:], w_blk(i, j), ident[:])
            nc.scalar.copy(wt_blk(j, i), ptr[:])

    if _gate(2):
        return
    # =========== phase V: means, centered support, mu ===========
    for b in range(NB):
        xgv = xg_sb[:, b * C * S:(b + 1) * C * S].rearrange("p (c s) -> p c s", s=S)
        mean_b = mean_sb[:, b * C:(b + 1) * C]
        nc.vector.tensor_reduce(mean_b, xgv, AX.X, OP.add)
        nc.vector.tensor_scalar(out=mean_b, in0=mean_b, scalar1=sc["inv_s"],
                                scalar2=None, op0=OP.mult)
        vv = vbuf[:, b * C * R:(b + 1) * C * R].rearrange("p (c r) -> p c r", r=R)
        nc.vector.tensor_sub(vv[:, :, 0:S], xgv,
                             mean_b[:, :, None].broadcast_to([B, C, S]))
        # column 16: sqrt(beta) * (mean - m)
        nc.vector.tensor_scalar(
            out=vv[:, :, S], in0=mean_b, scalar1=m_sb[:, b:b + 1],
            scalar2=sc["sbeta"], op0=OP.subtract, op1=OP.mult)
        nc.vector.memset(vv[:, :, S + 1], 0.0)
        # mu = cmu1*m + cmu2*mean  -> xmu_rhs[:, b*(QS+C)+QS : ...]
        mu_b = xmu_rhs[:, b * (QS + C) + QS: (b + 1) * (QS + C)]
        nc.vector.tensor_scalar(out=mu_b, in0=mean_b, scalar1=sc["cmu2"],
                                scalar2=None, op0=OP.mult)
        nc.vector.scalar_tensor_tensor(
            out=mu_b, in0=m_sb[:, b:b + 1].broadcast_to([B, C]),
            scalar=sc["cmu1"], in1=mu_b, op0=OP.mult, op1=OP.add)

    if _gate(3):
        return
    # =========== P = W @ V ===========
    NCH = 3
    CHW = C * R // NCH    # 384
    for i in range(NB):
        for ch in range(NCH):
            pp = psum([B, CHW])
            for k in range(i + 1):
                mmr(
                    pp[:], wt_blk(k, i),
                    vbuf[:, k * C * R + ch * CHW: k * C * R + (ch + 1) * CHW],
                    start=(k == 0), stop=(k == i))
            nc.scalar.copy(pbuf[:, i * C * R + ch * CHW: i * C * R + (ch + 1) * CHW], pp[:])

    if _gate(4):
        return
    # =========== t = W xq, u = W mu (fused: rhs = [xq | mu]) ===========
    W_RHS = QS + C
    for i in range(NB):
        pt = psum([B, W_RHS])
        for k in range(i + 1):
            mmr(pt[:], wt_blk(k, i),
                xmu_rhs[:, k * W_RHS:(k + 1) * W_RHS],
                start=(k == 0), stop=(k == i))
        nc.scalar.copy(tu_rhs[:, i * W_RHS:(i + 1) * W_RHS], pt[:])

    # =========== tu = u^T [t|u], xmu = mu^T [xq|mu] ===========
    ptu = psum([C, W_RHS])
    pxmu = psum([C, W_RHS])
    for k in range(NB):
        mmr(ptu[:], tu_rhs[:, k * W_RHS + QS:(k + 1) * W_RHS],
            tu_rhs[:, k * W_RHS:(k + 1) * W_RHS],
            start=(k == 0), stop=(k == NB - 1))
    for k in range(NB):
        mmr(pxmu[:], xmu_rhs[:, k * W_RHS + QS:(k + 1) * W_RHS],
            xmu_rhs[:, k * W_RHS:(k + 1) * W_RHS],
            start=(k == 0), stop=(k == NB - 1))
    nc.scalar.copy(tu_sb[:], ptu[:])
    nc.scalar.copy(xmu_sb[:], pxmu[:])
    nc.vector.tensor_mul(scr64[:], tu_sb[:, QS:], eyec[:])
    nc.vector.tensor_reduce(un_sb[:], scr64[:], AX.X, OP.add)
    nc.vector.tensor_mul(scr64[:], xmu_sb[:, QS:], eyec[:])
    nc.vector.tensor_reduce(mun_sb[:], scr64[:], AX.X, OP.add)

    # =========== squares + replicated row sums (tn | xn) ===========
    for b in range(NB):
        nc.scalar.square(t2x2[:, b * 2 * QS: b * 2 * QS + QS],
                         tu_rhs[:, b * W_RHS: b * W_RHS + QS])
        nc.scalar.square(t2x2[:, b * 2 * QS + QS:(b + 1) * 2 * QS],
                         xmu_rhs[:, b * W_RHS: b * W_RHS + QS])
    ptn = psum([C, 2 * QS])
    for b in range(NB):
        mmr(ptn[:], onesr[:], t2x2[:, b * 2 * QS:(b + 1) * 2 * QS],
            start=(b == 0), stop=(b == NB - 1))
    nc.scalar.copy(tnxn_sb[:], ptn[:])

    # =========== logdetA = sum log(td^2) (replicated to [C,1]) ===========
    nc.scalar.square(td_sb[:], td_sb[:])
    nc.scalar.activation(td_sb[:], td_sb[:], AF.Ln)
    plda = psum2([C, NB])
    nc.tensor.matmul(plda[:], onesr[:], td_sb[:], start=True, stop=True)
    nc.vector.tensor_reduce(lda_sb[:], plda[:], AX.X, OP.add)

    if _gate(5):
        return
    # =========== per-group: g = P_g^T [t|u] - b, M_g = P_g^T P_g ===========
    # M matmuls + extraction first so the Gauss-Jordan (DVE) overlaps the
    # whole PE scoring phase below.
    for g in range(NG):
        ncls = min(GC, C - g * GC)
        rows = ncls * R
        pM = psum([B, GC * R])
        for k in range(NB):
            lhs = pbuf[:, k * C * R + g * GC * R: k * C * R + g * GC * R + rows]
            nc.tensor.matmul(pM[:rows, :rows], lhs, lhs,
                             start=(k == 0), stop=(k == NB - 1))
        msc = spool.tile([B, GC * R], fp, name=f"msc{g}", tag="msc")
        nc.scalar.copy(msc[:rows, :rows], pM[:rows, :rows])
        for lc in range(ncls):
            cg = g * GC + lc
            dma_small(mbuf[cg:cg + 1, :].rearrange("p (i j) -> p i j", j=R),
                      msc[lc * R:(lc + 1) * R, lc * R:(lc + 1) * R])
    for g in range(NG):
        ncls = min(GC, C - g * GC)
        rows = ncls * R
        pg = psum([B, W_RHS])
        for k in range(NB):
            lhs = pbuf[:, k * C * R + g * GC * R: k * C * R + g * GC * R + rows]
            mmr(pg[:rows, :], lhs, tu_rhs[:, k * W_RHS:(k + 1) * W_RHS],
                start=(k == 0), stop=(k == NB - 1))
        # b_g[p] = sum_c (P_g^T u)[p, c] * maskb[p, c]
        bg = spool.tile([rows, 1], fp, name=f"bg{g}", tag="bg")
        bscr = spool.tile([B, C], fp, name=f"bscr{g}", tag="bscr")
        nc.vector.tensor_mul(bscr[:rows, :], pg[:rows, QS:],
                             maskb[:rows, g * C:(g + 1) * C])
        nc.vector.tensor_reduce(bg[:], bscr[:rows, :], AX.X, OP.add)
        nc.vector.tensor_scalar(out=gbuf[:rows, g * QS:(g + 1) * QS],
                                in0=pg[:rows, 0:QS], scalar1=bg[:],
                                scalar2=None, op0=OP.subtract)

    if _gate(6):
        return
    # =========== batched Gauss-Jordan on mbuf [C, R*R] ===========
    nc.vector.tensor_add(mbuf[:], mbuf[:], eyeflat[:])
    mview = mbuf.rearrange("p (i j) -> p i j", j=R)
    nc.vector.memset(logpiv[:, R - 1:], 0.0)
    for k in range(R - 1):
        pv = mbuf[:, k * (R + 1): k * (R + 1) + 1]
        rp = spool.tile([C, 1], fp, name=f"rp{k}", tag="rp")
        rowk = spool.tile([C, R], fp, name=f"rowk{k}", tag="rowk")
        colk = spool.tile([C, R], fp, name=f"colk{k}", tag="colk")
        tmpo = spool.tile([C, R, R], fp, name=f"tmpo{k}", tag="tmpo")
        nc.scalar.activation(logpiv[:, k: k + 1], pv, AF.Ln)
        nc.vector.reciprocal(rp[:], pv)
        nc.vector.tensor_scalar(out=rowk[:], in0=mview[:, k, :], scalar1=rp[:],
                                scalar2=None, op0=OP.mult)
        nc.gpsimd.tensor_copy(colk[:], mview[:, :, k])
        nc.vector.tensor_mul(
            tmpo[:],
            colk[:, :, None].broadcast_to([C, R, R]),
            rowk[:, None, :].broadcast_to([C, R, R]))
        nc.vector.tensor_sub(mbuf[:], mbuf[:], tmpo.rearrange("p i j -> p (i j)"))
        nc.gpsimd.tensor_copy(mview[:, k, :], rowk[:])
        nc.gpsimd.tensor_scalar(out=mview[:, :, k], in0=colk[:], scalar1=rp[:],
                                scalar2=-1.0, op0=OP.mult, op1=OP.mult)
        nc.gpsimd.tensor_copy(pv, rp[:])
    nc.vector.tensor_reduce(ld2_sb[:], logpiv[:], AX.X, OP.add)
    # bias = BC0 - 0.5*(logdetM + logdetA)
    nc.vector.tensor_add(bias_sb[:], ld2_sb[:], lda_sb[:])
    nc.vector.tensor_scalar(out=bias_sb[:], in0=bias_sb[:], scalar1=-0.5,
                            scalar2=sc["BC0"], op0=OP.mult, op1=OP.add)

    if _gate(7):
        return
    # =========== block-diag K, h = K g, corr ===========
    for g in range(NG):
        ncls = min(GC, C - g * GC)
        for lc in range(ncls):
            cg = g * GC + lc
            dma_small(kdfull[lc * R:(lc + 1) * R,
                             g * GC * R + lc * R: g * GC * R + (lc + 1) * R],
                      mbuf[cg:cg + 1, :].rearrange("p (i j) -> p i j", j=R))
    for g in range(NG):
        ncls = min(GC, C - g * GC)
        rows = ncls * R
        ph = psum([B, QS])
        mmr(ph[:rows, :], kdfull[0:rows, g * GC * R: g * GC * R + rows],
            gbuf[0:rows, g * QS:(g + 1) * QS], start=True, stop=True)
        prod = spool.tile([B, QS], fp, name=f"prod{g}", tag="prod")
        nc.vector.tensor_mul(prod[:rows, :], ph[:rows, :], gbuf[0:rows, g * QS:(g + 1) * QS])
        pc = psum2([GC, QS])
        mmr(pc[:ncls, :], maski[0:rows, g * GC: g * GC + ncls],
            prod[:rows, :], start=True, stop=True)
        csc = spool.tile([GC, QS], fp, name=f"csc{g}", tag="csc")
        nc.scalar.copy(csc[:ncls, :], pc[:ncls, :])
        dma(corrbuf[g * GC: g * GC + ncls, :], csc[:ncls, :])

    if _gate(8):
        return
    # =========== assemble logits ===========
    wda = spool.tile([C, QS], fp, name="wda", tag="wda", bufs=1)
    d2a = spool.tile([C, QS], fp, name="d2a", tag="d2a", bufs=1)
    acc = spool.tile([C, QS], fp, name="acc", tag="acc", bufs=1)
    # wd2 = tn - 2*tu + un
    nc.vector.scalar_tensor_tensor(out=wda[:], in0=tu_sb[:, 0:QS], scalar=-2.0,
                                   in1=tnxn_sb[:, 0:QS], op0=OP.mult, op1=OP.add)
    nc.vector.tensor_scalar(out=wda[:], in0=wda[:], scalar1=un_sb[:],
                            scalar2=None, op0=OP.add)
    # d2 + mun + common/REG
    nc.vector.scalar_tensor_tensor(out=d2a[:], in0=xmu_sb[:, 0:QS], scalar=-2.0,
                                   in1=tnxn_sb[:, QS:], op0=OP.mult, op1=OP.add)
    nc.vector.tensor_scalar(out=d2a[:], in0=d2a[:], scalar1=mun_sb[:],
                            scalar2=sc["common"] / REG, op0=OP.add, op1=OP.add)
    # acc = ca*(wd2 - corr) + REG*d2' = maha + common
    nc.vector.tensor_sub(acc[:], wda[:], corrbuf[:])
    nc.vector.tensor_scalar(out=acc[:], in0=acc[:], scalar1=sc["ca"],
                            scalar2=None, op0=OP.mult)
    nc.vector.scalar_tensor_tensor(out=acc[:], in0=d2a[:], scalar=REG,
                                   in1=acc[:], op0=OP.mult, op1=OP.add)
    nc.scalar.activation(acc[:], acc[:], AF.Ln)
    nc.vector.tensor_scalar(out=logits_sb[:], in0=acc[:], scalar1=-sc["CC"],
                            scalar2=bias_sb[:], op0=OP.mult, op1=OP.add)
    dma(ins["out"][:], logits_sb[:])

    for p in (ps2, ps, spool, pool):
        p.release()


def build_program(sc):
    import concourse.mybir as mybir
    import concourse.tile as tile
    from concourse import bacc

    nc = bacc.Bacc("TRN2", target_bir_lowering=False, debug=False,
                   num_devices=NCORES)
    fp = mybir.dt.float32
    shapes = dict(
        xgt=[D, C * S], ltf=[D, D], ltds=[D, B], xqt=[D, QS],
        mcol=[D, 1], tdcol=[D, 1], ident=[B, B], onesr=[B, C],
        eyec=[C, C], eyeflat=[C, R * R], maskb=[B, NG * C], maski=[B, NG * GC],
    )
    ins = {name: nc.dram_tensor(name, shp, fp, kind="ExternalInput").ap()
           for name, shp in shapes.items()}
    ins["out"] = nc.dram_tensor("out", [C, QS], fp, kind="ExternalOutput").ap()
    with tile.TileContext(nc) as tc:
        _emit(nc, tc, ins, sc)
    nc.compile()
    return nc


_BUILD_CACHE = {}


def kernel(**inputs) -> np.ndarray:
    from concourse import bass_utils

    shared, xqts, sc = _host_prep(inputs)
    key = tuple(sorted(sc.items()))
    if key not in _BUILD_CACHE:
        _BUILD_CACHE[key] = build_program(sc)
    nc = _BUILD_CACHE[key]

    in_maps = []
    for c in range(NCORES):
        im = {k: v for k, v in shared.items()}
        im["xqt"] = xqts[c]
        in_maps.append(im)
    res = bass_utils.run_bass_kernel_spmd(nc, in_maps, core_ids=list(range(NCORES)))
    logits = np.concatenate([r["out"].T for r in res.results], axis=0)
    return logits.astype(np.float32)


if __name__ == "__main__":
    rng = np.random.default_rng(0)
    demo = dict(
        X_support=rng.standard_normal((C * S, D), dtype=np.float32),
        y=np.repeat(np.arange(C, dtype=np.int64), S),
        X_query=rng.standard_normal((Q, D), dtype=np.float32),
        m=0.01 * rng.standard_normal((1, D)).astype(np.float32),
        kappa=np.float32(0.1), nu=np.float32(D),
        triu_diag=np.ones(D, dtype=np.float32),
        triu_lower=(np.eye(D) + 0.01 * rng.standard_normal((D, D))).astype(np.float32),
    )
    out = kernel(**demo)
    print(out.shape, out.dtype, np.abs(out).max())



# revision 17
# speedup vs baseline: 3.4281x; 3.4281x over previous
"""Trainium2 Bass kernel for nn_MetaQDA_FB (MetaQDA Fisher-Bayes logits).

Math: sigma_c = scale * (L L^T + V_c V_c^T).  The 16 centered shots are
host-projected onto a fixed orthonormal basis U of 1-perp (Y = Xg U, exact
since 1^T U = 0), giving V_c = [Y_c, sqrt(beta)(mean_c - m)] of rank R=16,
so 8 classes pack exactly into 128 partitions (NG=8 groups of GC=8).

Per-class inversion/logdet uses a shared triangular inverse W = L^{-1}
(blocked degree-3 Neumann-by-squaring on the diagonal + forward
substitution, all bf16 matmuls) plus 16x16 capacitance matrices
M_c = I + (W V_c)^T (W V_c).  M_c^{-1} is approximated on the PE with a
Jacobi-preconditioned Neumann series: J = rsd (M - D) rsd (||J|| ~ 0.4),
S = (I - J)(I + J^2) per 8-class group as dense [128,128] bf16 matmuls,
logdet M = sum ln(diag) - tr(J^2)/2.

Queries are sharded across the 8 cores (256 each); every core redundantly
builds the (cheap) per-class data and scores its own query block:

  maha_qc = (1-REG)/scale * (||W(x_q-mu_c)||^2 - g^T K_c g) + REG ||x_q-mu_c||^2
  logits  = bias_c - 0.5(common+d) * ln(common + maha)

Host does input reordering and linear prep (sort, U-projection, means, mu,
L-block extraction); all O(n^3) compute runs on device.
"""

import math
import sys

import numpy as np

for _p in ("/opt/trn_rl_repo",):
    if _p not in sys.path:
        sys.path.append(_p)

D, C, S, Q, REG, EPS = 640, 64, 16, 2048, 0.3, 1e-6
B = 128
NB = D // B            # 5 row/col blocks of L
R = 16                 # rank per class after U-projection
GC = 8                 # classes per group (GC*R = 128)
NG = C // GC           # 8 groups
NCORES = 8
QS = Q // NCORES       # queries per core
WR = QS + C            # fused [t | u] rhs width
CR = C * R             # 1024
NLT = NB * (NB - 1) // 2   # strict-upper LT block pairs
STOP_AFTER = 99        # debug: truncate kernel after phase N
DEBUG_DUMP = False     # dump intermediates as extra outputs
F32 = np.float32


def _bf16(x):
    import ml_dtypes
    return np.asarray(x, dtype=F32).astype(ml_dtypes.bfloat16)


def _lt_pairs():
    return [(k, i) for k in range(NB) for i in range(k + 1, NB)]


def _host_prep(inputs):
    Xs = np.asarray(inputs["X_support"], dtype=np.float64)
    y = np.asarray(inputs["y"])
    Xq = np.asarray(inputs["X_query"], dtype=F32)
    m = np.asarray(inputs["m"], dtype=np.float64).reshape(-1)
    kappa = float(np.asarray(inputs["kappa"]))
    nu = float(np.asarray(inputs["nu"]))
    td = np.asarray(inputs["triu_diag"], dtype=np.float64).reshape(-1)
    tl = np.asarray(inputs["triu_lower"], dtype=np.float64)

    perm = np.argsort(y, kind="stable")
    Xg = Xs[perm].reshape(C, S, D)

    mask = np.tril(np.ones((D, D)), k=-1)
    L = np.diag(np.abs(td)) + tl * mask
    LT = L.T

    kappa_ = abs(kappa) + EPS
    nu_ = max(nu, D - 1 + EPS)
    Nj = float(S)
    scale = (kappa_ + Nj + 1.0) / ((nu_ + Nj - D + 1.0) * (kappa_ + Nj))
    common = nu_ + Nj + 1.0 - D
    beta = kappa_ * Nj / (kappa_ + Nj)
    lda = float(np.sum(np.log(td ** 2)))
    BC0 = (math.lgamma(0.5 * (common + D)) - math.lgamma(0.5 * common)
           - 0.5 * D * math.log(common)
           - 0.5 * D * math.log(scale)
           - 0.5 * lda
           + 0.5 * (common + D) * math.log(common))
    sc = dict(
        common=common, ca=(1.0 - REG) / scale,
        BC0=BC0, CC=0.5 * (common + D),
    )

    # U: orthonormal basis of 1-perp in R^S  (fixed, exact to fp32)
    Ac = np.eye(S) - np.ones((S, S)) / S
    Uq, _ = np.linalg.qr(Ac)
    U15 = Uq[:, :S - 1]                                       # [16, 15]

    mean = Xg.mean(axis=1)                                    # [C, D]
    mu = (kappa_ / (kappa_ + Nj)) * m + (Nj / (kappa_ + Nj)) * mean
    XgU = np.einsum('csd,st->cdt', Xg, U15)                   # [C, D, 15]
    v_host = np.zeros((D, C * R), dtype=np.float64)           # [D, (c r)]
    for c in range(C):
        v_host[:, c * R:c * R + (S - 1)] = XgU[c]
        v_host[:, c * R + (S - 1)] = math.sqrt(beta) * (mean[c] - m)

    # E blocks (negated strict lower of diag blocks), T1 = I + E
    ebf = np.zeros((B, NB * B))
    etbf = np.zeros((B, NB * B))
    ltp = np.zeros((B, NLT * B))
    for b in range(NB):
        Lbb = L[b * B:(b + 1) * B, b * B:(b + 1) * B]
        E = -np.tril(Lbb, -1)
        ebf[:, b * B:(b + 1) * B] = E
        etbf[:, b * B:(b + 1) * B] = E.T
    for j, (k, i) in enumerate(_lt_pairs()):
        ltp[:, j * B:(j + 1) * B] = LT[k * B:(k + 1) * B, i * B:(i + 1) * B]

    # constants
    ident_bf = np.eye(B)
    twoi_bf = 2.0 * np.eye(B)
    ident32 = np.eye(B, dtype=F32)
    ones32 = np.ones((B, 1), dtype=F32)
    onesr_bf = np.ones((B, C))
    clsid_bf = np.zeros((GC, B))          # [q, p] = 1 iff p//R == q
    clsidT32 = np.zeros((B, GC), dtype=F32)
    for p in range(B):
        clsid_bf[p // R, p] = 1.0
        clsidT32[p, p // R] = 1.0
    maskb = np.zeros((B, NG * C), dtype=F32)
    for g in range(NG):
        for p in range(B):
            maskb[p, g * C + g * GC + p // R] = 1.0

    shared = dict(
        ebf=_bf16(ebf), etbf=_bf16(etbf), ltp=_bf16(ltp),
        vb=_bf16(v_host), mub=_bf16(mu.T),
        ident_bf=_bf16(ident_bf), twoi_bf=_bf16(twoi_bf),
        clsid_bf=_bf16(clsid_bf),
        ident32=ident32, ones32=ones32, onesr_bf=_bf16(onesr_bf),
        clsidT32=clsidT32, maskb=maskb, maskb_bf=_bf16(maskb),
    )
    xqts = [_bf16(np.ascontiguousarray(Xq[c * QS:(c + 1) * QS].T))
            for c in range(NCORES)]
    return shared, xqts, sc


def _emit(nc, tc, ins, sc):
    import concourse.mybir as mybir

    fp = mybir.dt.float32
    bf = mybir.dt.bfloat16
    fpr = mybir.dt.float32r
    AF = mybir.ActivationFunctionType
    OP = mybir.AluOpType
    AX = mybir.AxisListType

    pool = tc.alloc_tile_pool(name="persist", bufs=1)
    spool = tc.alloc_tile_pool(name="scratch", bufs=2)
    psA = tc.alloc_tile_pool(name="psA", bufs=2, space="PSUM")   # big stream
    psM = tc.alloc_tile_pool(name="psM", bufs=2, space="PSUM")   # M groups
    psN = tc.alloc_tile_pool(name="psN", bufs=3, space="PSUM")   # NK smalls
    psC = tc.alloc_tile_pool(name="psC", bufs=1, space="PSUM")   # corr accum

    mm = nc.tensor.matmul

    def mmr(out, lhsT, rhs, **kw):
        mm(out, lhsT.bitcast(fpr), rhs.bitcast(fpr), **kw)

    dma = nc.sync.dma_start
    _dma_engines = [nc.sync, nc.gpsimd, nc.scalar]
    _dma_rr = [0]

    def dma_rr(out, in_):
        eng = _dma_engines[_dma_rr[0] % len(_dma_engines)]
        _dma_rr[0] += 1
        eng.dma_start(out, in_)

    # psum -> sbuf cast/copy round-robin (DVE / Act only: both read PSUM)
    _cast_rr = [0]

    def cast(out, in_):
        if _cast_rr[0] % 2 == 0:
            nc.vector.tensor_copy(out, in_)
        else:
            nc.scalar.copy(out, in_)
        _cast_rr[0] += 1

    # ---- persistent SBUF tensors ----
    def T(name, shape, dt=fp):
        return pool.tile(shape, dt, name=name, tag=name)

    ebf = T("ebf", [B, NB * B], bf)
    etbf = T("etbf", [B, NB * B], bf)
    t1bf = T("t1bf", [B, NB * B], bf)
    t1tbf = T("t1tbf", [B, NB * B], bf)
    ltp = T("ltp", [B, NLT * B], bf)
    wbf = T("wbf", [B, NB * D], bf)        # W block (i,j) at i*D+j*B
    wtbf = T("wtbf", [B, NB * D], bf)      # W^T block (a,b) at a*D+b*B
    vbuf = T("vbuf", [B, NB * CR], bf)
    xmu = T("xmu", [B, NB * WR], bf)       # [xq | mu] per k block
    tubf = T("tubf", [B, NB * WR], bf)     # [t | u] per i block
    t2x2 = T("t2x2", [B, NB * 2 * QS], bf)
    pbuf = T("pbuf", [B, NB * CR], bf)
    s2sb = T("s2sb", [B, NG * B], bf)      # per-group Neumann core S
    gbuf = T("gbuf", [B, NG * QS], bf)
    rsd32 = T("rsd32", [B, NG])
    dmsb = T("dmsb", [B, NG])
    trln = T("trln", [B, 2 * NG])          # [tr2 cols | lnD cols]
    tusb = T("tusb", [C, WR])
    xmusb = T("xmusb", [C, WR])
    tnxn = T("tnxn", [C, 2 * QS])
    un_sb = T("un_sb", [C, 1])
    mun_sb = T("mun_sb", [C, 1])
    bias_sb = T("bias_sb", [C, 1])
    logits_sb = T("logits_sb", [C, QS])
    scr64 = T("scr64", [C, C])
    # consts
    ident_bf = T("ident_bf", [B, B], bf)
    twoi_bf = T("twoi_bf", [B, B], bf)
    clsid_bf = T("clsid_bf", [GC, B], bf)
    ident32 = T("ident32", [B, B])
    ones32 = T("ones32", [B, 1])
    onesr_bf = T("onesr_bf", [B, C], bf)
    clsidT32 = T("clsidT32", [B, GC])
    maskb = T("maskb", [B, NG * C])
    maskb_bf = T("maskb_bf", [B, NG * C], bf)

    # ---- input DMAs (W-phase inputs first; spread queues) ----
    dma(ident_bf[:], ins["ident_bf"][:])
    dma(ebf[:], ins["ebf"][:])
    nc.gpsimd.dma_start(etbf[:], ins["etbf"][:])
    nc.scalar.dma_start(ltp[:], ins["ltp"][:])
    nc.gpsimd.dma_start(vbuf.rearrange("p (b n) -> p b n", b=NB),
                        ins["vb"].rearrange("(b p) n -> p b n", p=B))
    dma(xmu.rearrange("p (b w) -> p b w", w=WR)[:, :, 0:QS],
        ins["xqt"].rearrange("(b p) n -> p b n", p=B))
    nc.scalar.dma_start(xmu.rearrange("p (b w) -> p b w", w=WR)[:, :, QS:],
                        ins["mub"].rearrange("(b p) c -> p b c", p=B))
    for cname, ct in (("twoi_bf", twoi_bf), ("clsid_bf", clsid_bf),
                      ("ident32", ident32), ("ones32", ones32),
                      ("onesr_bf", onesr_bf), ("clsidT32", clsidT32),
                      ("maskb", maskb), ("maskb_bf", maskb_bf)):
        dma_rr(ct[:], ins[cname][:])

    # zero the strictly-upper W blocks (read as zeros in fwd substitution)
    for k in range(NB - 1):
        nc.gpsimd.memset(wbf[:, k * D + (k + 1) * B: (k + 1) * D], 0.0)

    # T1 = I + E (broadcast identity across the 5 blocks)
    ib = ident_bf[:, None, :].broadcast_to([B, NB, B])
    nc.vector.tensor_add(t1bf.rearrange("p (b n) -> p b n", b=NB),
                         ebf.rearrange("p (b n) -> p b n", b=NB), ib)
    nc.vector.tensor_add(t1tbf.rearrange("p (b n) -> p b n", b=NB),
                         etbf.rearrange("p (b n) -> p b n", b=NB), ib)

    def _gate(n):
        if STOP_AFTER <= n:
            nc.vector.memset(logits_sb[:], 0.0)
            dma(ins["out"][:], logits_sb[:])
            for p in (psC, psN, psM, psA, spool, pool):
                p.release()
            return True
        return False

    eb = lambda b: ebf[:, b * B:(b + 1) * B]
    etb = lambda b: etbf[:, b * B:(b + 1) * B]
    t1b = lambda b: t1bf[:, b * B:(b + 1) * B]
    t1tb = lambda b: t1tbf[:, b * B:(b + 1) * B]
    w_blk = lambda i, j: wbf[:, i * D + j * B: i * D + (j + 1) * B]
    wt_blk = lambda a, b: wtbf[:, a * D + b * B: a * D + (b + 1) * B]
    _ltidx = {ki: j for j, ki in enumerate(_lt_pairs())}
    lt_blk = lambda k, i: ltp[:, _ltidx[(k, i)] * B:(_ltidx[(k, i)] + 1) * B]

    # =========== phase W-diag: W_bb = (I+E)(I+E^2), deg-3 Neumann ===========
    for b in range(NB):
        e2tps = psN.tile([B, B], fp, name=f"e2tps{b}", tag="nks")
        mm(e2tps[:], eb(b), etb(b), start=True, stop=True)
        e2tbf = spool.tile([B, B], bf, name=f"e2tbf{b}", tag="e2tbf", bufs=3)
        cast(e2tbf[:], e2tps[:])
        wps = psN.tile([B, B], fp, name=f"wps{b}", tag="nks")
        mm(wps[:], e2tbf[:], t1b(b), start=True, stop=False)
        mm(wps[:], ident_bf[:], t1b(b), start=False, stop=True)
        cast(w_blk(b, b), wps[:])
        wtps = psN.tile([B, B], fp, name=f"wtps{b}", tag="nks")
        mm(wtps[:], t1b(b), e2tbf[:], start=True, stop=False)
        mm(wtps[:], ident_bf[:], t1tb(b), start=False, stop=True)
        cast(wt_blk(b, b), wtps[:])

    # preload the Sqrt activation table off the critical path
    akscr = spool.tile([1, 1], fp, name="akscr", tag="akscr", bufs=1)
    nc.scalar.activation(akscr[:], ones32[0:1, 0:1], AF.Sqrt)

    if _gate(1):
        return
    # =========== phase W-offdiag: row-batched forward substitution ===========
    for i in range(1, NB):
        accps = psA.tile([B, i * B], fp, name=f"acc{i}", tag="bigA")
        for k in range(i):
            mm(accps[:], lt_blk(k, i), wbf[:, k * D: k * D + i * B],
               start=(k == 0), stop=(k == i - 1))
        tij = spool.tile([B, i * B], bf, name=f"tij{i}", tag="tij", bufs=3)
        cast(tij[:], accps[:])
        wps2 = psA.tile([B, i * B], fp, name=f"wo{i}", tag="bigA")
        mm(wps2[:], wt_blk(i, i), tij[:], start=True, stop=True)
        nc.vector.tensor_scalar(out=wbf[:, i * D: i * D + i * B], in0=wps2[:],
                                scalar1=-1.0, scalar2=None, op0=OP.mult)
        for j in range(i):
            trps = psN.tile([B, B], bf, name=f"tr{i}{j}", tag="nks")
            nc.tensor.transpose(trps[:], w_blk(i, j), ident_bf[:])
            cast(wt_blk(j, i), trps[:])

    if _gate(2):
        return
    # =========== P = W @ V (bf16) ===========
    for i in range(NB):
        for ch in range(2):
            pps = psA.tile([B, CR // 2], fp, name=f"p{i}{ch}", tag="bigA")
            for k in range(i + 1):
                mm(pps[:], wt_blk(k, i),
                   vbuf[:, k * CR + ch * (CR // 2): k * CR + (ch + 1) * (CR // 2)],
                   start=(k == 0), stop=(k == i))
            cast(pbuf[:, i * CR + ch * (CR // 2): i * CR + (ch + 1) * (CR // 2)],
                 pps[:])

    if _gate(3):
        return
    # =========== t = W xq, u = W mu (fused rhs = [xq | mu]) ===========
    for i in range(NB):
        tups = psA.tile([B, WR], fp, name=f"tu{i}", tag="bigA")
        for k in range(i + 1):
            mm(tups[:], wt_blk(k, i), xmu[:, k * WR:(k + 1) * WR],
               start=(k == 0), stop=(k == i))
        nc.vector.tensor_copy(tubf[:, i * WR:(i + 1) * WR], tups[:])
        nc.scalar.square(t2x2[:, i * 2 * QS: i * 2 * QS + QS], tups[:, 0:QS])
        nc.scalar.square(t2x2[:, i * 2 * QS + QS:(i + 1) * 2 * QS],
                         xmu[:, i * WR: i * WR + QS])

    # =========== tu = u^T [t|u], xmu = mu^T [xq|mu] ===========
    ptu = psA.tile([C, WR], fp, name="ptu", tag="bigA")
    for k in range(NB):
        mm(ptu[:], tubf[:, k * WR + QS:(k + 1) * WR],
           tubf[:, k * WR:(k + 1) * WR], start=(k == 0), stop=(k == NB - 1))
    nc.scalar.copy(tusb[:], ptu[:])
    pxmu = psA.tile([C, WR], fp, name="pxmu", tag="bigA")
    for k in range(NB):
        mm(pxmu[:], xmu[:, k * WR + QS:(k + 1) * WR],
           xmu[:, k * WR:(k + 1) * WR], start=(k == 0), stop=(k == NB - 1))
    nc.scalar.copy(xmusb[:], pxmu[:])
    nc.vector.tensor_mul(scr64[:], tusb[:, QS:], ident32[0:C, 0:C])
    nc.vector.tensor_reduce(un_sb[:], scr64[:], AX.X, OP.add)
    nc.vector.tensor_mul(scr64[:], xmusb[:, QS:], ident32[0:C, 0:C])
    nc.vector.tensor_reduce(mun_sb[:], scr64[:], AX.X, OP.add)

    # =========== tn | xn row sums (replicated over classes) ===========
    ptn = psA.tile([C, 2 * QS], fp, name="ptn", tag="bigA")
    for bk in range(NB):
        mm(ptn[:], onesr_bf[:], t2x2[:, bk * 2 * QS:(bk + 1) * 2 * QS],
           start=(bk == 0), stop=(bk == NB - 1))
    nc.scalar.copy(tnxn[:], ptn[:])

    if _gate(4):
        return
    # =========== M_g = P_g^T P_g  (two [B,512] psum tiles, 4 groups each) ===========
    mts = [psM.tile([B, 4 * B], fp, name=f"mts{h}", tag="psM") for h in range(2)]
    mreg = lambda g: mts[g // 4][:, (g % 4) * B:(g % 4 + 1) * B]
    for g in range(NG):
        pslc = lambda k: pbuf[:, k * CR + g * B: k * CR + (g + 1) * B]
        for k in range(NB):
            mm(mreg(g), pslc(k), pslc(k), start=(k == 0), stop=False)
        mm(mreg(g), ident_bf[:], ident_bf[:], start=False, stop=True)  # M = I + P^T P

    # =========== NK feeds: diag, rsd, masked col-scale, J, I-J ===========
    mi_l, jbf_l, imj_l = [], [], []
    for g in range(NG):
        mi = spool.tile([B, B], fp, name=f"mi{g}", tag="mi", bufs=NG)
        nc.vector.tensor_mul(mi[:], mreg(g), ident32[:])
        mi_l.append(mi)
    dm_l = []
    for g in range(NG):
        dmps = psN.tile([B, 1], fp, name=f"dm{g}", tag="nks")
        mm(dmps[:], mi_l[g][:], ones32[:], start=True, stop=True)
        dm_l.append(dmps)
    rsdbf_l = []
    for g in range(NG):
        nc.vector.tensor_copy(dmsb[:, g:g + 1], dm_l[g][:])
        rcp = spool.tile([B, 1], fp, name=f"rcp{g}", tag="rcp", bufs=NG)
        nc.vector.reciprocal(rcp[:], dm_l[g][:])
        rsdbf = spool.tile([B, 1], bf, name=f"rsdb{g}", tag="rsdb", bufs=NG)
        nc.scalar.activation(rsdbf[:], rcp[:], AF.Sqrt)
        rsdbf_l.append(rsdbf)
    spr_l = []
    for g in range(NG):
        nc.vector.tensor_copy(rsd32[:, g:g + 1], rsdbf_l[g][:])
        spr = spool.tile([B, GC], bf, name=f"spr{g}", tag="spr", bufs=NG)
        nc.vector.tensor_mul(
            spr[:], rsd32[:, g:g + 1].broadcast_to([B, GC]), clsidT32[:])
        spr_l.append(spr)
    sps_l = []
    for g in range(NG):
        spps = psN.tile([GC, B], bf, name=f"spp{g}", tag="nks")
        nc.tensor.transpose(spps[:], spr_l[g][:], ident_bf[:])
        spsb = spool.tile([GC, B], bf, name=f"sps{g}", tag="sps", bufs=NG)
        nc.scalar.copy(spsb[:], spps[:])
        sps_l.append(spsb)
    rmf_l = []
    for g in range(NG):
        rmfps = psN.tile([B, B], fp, name=f"rmf{g}", tag="nks")
        mm(rmfps[:], clsid_bf[:], sps_l[g][:], start=True, stop=True)
        rmf = spool.tile([B, B], fp, name=f"rmfs{g}", tag="rmfs", bufs=NG)
        nc.scalar.copy(rmf[:], rmfps[:])
        rmf_l.append(rmf)
    for g in range(NG):
        jraw = spool.tile([B, B], bf, name=f"jraw{g}", tag="jraw", bufs=NG)
        nc.vector.scalar_tensor_tensor(
            out=jraw[:], in0=mreg(g), scalar=rsd32[:, g:g + 1], in1=rmf_l[g][:],
            op0=OP.mult, op1=OP.mult)
        jbf = spool.tile([B, B], bf, name=f"jbf{g}", tag="jbf", bufs=NG)
        nc.vector.tensor_sub(jbf[:], jraw[:], ident_bf[:])
        jbf_l.append(jbf)
        imj = spool.tile([B, B], bf, name=f"imj{g}", tag="imj", bufs=NG)
        nc.gpsimd.tensor_sub(imj[:], twoi_bf[:], jraw[:])
        imj_l.append(imj)

    if _gate(5):
        return
    # =========== g = P_g^T [t|u] - b, scaled by rsd ===========
    for g in range(NG):
        pg = psA.tile([B, WR], fp, name=f"pg{g}", tag="bigA")
        for k in range(NB):
            mm(pg[:], pbuf[:, k * CR + g * B: k * CR + (g + 1) * B],
               tubf[:, k * WR:(k + 1) * WR], start=(k == 0), stop=(k == NB - 1))
        bscr = spool.tile([B, C], fp, name=f"bscr{g}", tag="bscr", bufs=4)
        nc.vector.tensor_mul(bscr[:], pg[:, QS:], maskb[:, g * C:(g + 1) * C])
        bg = spool.tile([B, 1], fp, name=f"bg{g}", tag="bg", bufs=4)
        nc.vector.tensor_reduce(bg[:], bscr[:], AX.X, OP.add)
        nc.vector.tensor_scalar(out=gbuf[:, g * QS:(g + 1) * QS],
                                in0=pg[:, 0:QS], scalar1=bg[:],
                                scalar2=rsd32[:, g:g + 1],
                                op0=OP.subtract, op1=OP.mult)

    if _gate(6):
        return
    # =========== NK matmuls: S_g = (I + J^2)(I - J) ===========
    for g in range(NG):
        j2ps = psN.tile([B, B], fp, name=f"j2{g}", tag="nks")
        mm(j2ps[:], jbf_l[g][:], jbf_l[g][:], start=True, stop=True)
        j2bf = spool.tile([B, B], bf, name=f"j2b{g}", tag="j2b", bufs=NG)
        cast(j2bf[:], j2ps[:])
        s1ps = psN.tile([B, B], fp, name=f"s1{g}", tag="nks")
        mm(s1ps[:], j2bf[:], imj_l[g][:], start=True, stop=False)
        mm(s1ps[:], ident_bf[:], imj_l[g][:], start=False, stop=True)
        cast(s2sb[:, g * B:(g + 1) * B], s1ps[:])
        sq = spool.tile([B, B], fp, name=f"sq{g}", tag="sq", bufs=4)
        nc.scalar.square(sq[:], jbf_l[g][:])
        t2g = psN.tile([B, 1], fp, name=f"t2g{g}", tag="nks")
        mm(t2g[:], sq[:], ones32[:], start=True, stop=True)
        nc.vector.tensor_copy(trln[:, g:g + 1], t2g[:])

    # lnD after all Rsqrt ops (one act-table switch)
    nc.scalar.activation(trln[:, NG:], dmsb[:], AF.Ln)

    if _gate(7):
        return
    # =========== Kh, corr (two accumulators), class-summed bias ===========
    corrA = psC.tile([C, QS], fp, name="corrA", tag="corrA")
    corrB = psM.tile([C, QS], fp, name="corrB", tag="psM")
    for g in range(NG):
        hps = psA.tile([B, QS], fp, name=f"h{g}", tag="bigA")
        mm(hps[:], s2sb[:, g * B:(g + 1) * B], gbuf[:, g * QS:(g + 1) * QS],
           start=True, stop=True)
        prod = spool.tile([B, QS], bf, name=f"prod{g}", tag="prod", bufs=NG)
        nc.vector.tensor_mul(prod[:], hps[:], gbuf[:, g * QS:(g + 1) * QS])
        tgt = corrA if g % 2 == 0 else corrB
        mm(tgt[:], maskb_bf[:, g * C:(g + 1) * C], prod[:],
           start=(g < 2), stop=(g >= NG - 2))

    clsps = psN.tile([C, 2], fp, name="clsps", tag="nks")
    trv = trln.rearrange("p (two g) -> p g two", g=NG)
    for g in range(NG):
        mm(clsps[:], maskb[:, g * C:(g + 1) * C], trv[:, g, :],
           start=(g == 0), stop=(g == NG - 1))
    nc.vector.tensor_scalar(out=bias_sb[:], in0=clsps[:, 1:2], scalar1=-0.5,
                            scalar2=sc["BC0"], op0=OP.mult, op1=OP.add)
    nc.vector.scalar_tensor_tensor(out=bias_sb[:], in0=clsps[:, 0:1],
                                   scalar=0.25, in1=bias_sb[:],
                                   op0=OP.mult, op1=OP.add)

    if _gate(8):
        return
    # =========== assemble logits ===========
    wda = spool.tile([C, QS], fp, name="wda", tag="wda", bufs=1)
    d2a = spool.tile([C, QS], fp, name="d2a", tag="d2a", bufs=1)
    acc = spool.tile([C, QS], fp, name="acc", tag="acc", bufs=1)
    # wd2 = tn - 2*tu + un
    nc.vector.scalar_tensor_tensor(out=wda[:], in0=tusb[:, 0:QS], scalar=-2.0,
                                   in1=tnxn[:, 0:QS], op0=OP.mult, op1=OP.add)
    nc.vector.tensor_scalar(out=wda[:], in0=wda[:], scalar1=un_sb[:],
                            scalar2=None, op0=OP.add)
    # d2 + mun + common/REG
    nc.vector.scalar_tensor_tensor(out=d2a[:], in0=xmusb[:, 0:QS], scalar=-2.0,
                                   in1=tnxn[:, QS:], op0=OP.mult, op1=OP.add)
    nc.vector.tensor_scalar(out=d2a[:], in0=d2a[:], scalar1=mun_sb[:],
                            scalar2=sc["common"] / REG, op0=OP.add, op1=OP.add)
    # acc = ca*(wd2 - corrA - corrB) + REG*d2' = maha + common
    nc.vector.tensor_sub(acc[:], wda[:], corrA[:])
    nc.vector.tensor_sub(acc[:], acc[:], corrB[:])
    nc.vector.tensor_scalar(out=acc[:], in0=acc[:], scalar1=sc["ca"],
                            scalar2=None, op0=OP.mult)
    nc.vector.scalar_tensor_tensor(out=acc[:], in0=d2a[:], scalar=REG,
                                   in1=acc[:], op0=OP.mult, op1=OP.add)
    if DEBUG_DUMP:
        corrs = spool.tile([C, QS], fp, name="corrs", tag="corrs", bufs=1)
        nc.vector.tensor_add(corrs[:], corrA[:], corrB[:])
        for nm, t in (("d_rsd32", rsd32), ("d_dmsb", dmsb), ("d_trln", trln),
                      ("d_bias", bias_sb), ("d_s2", s2sb), ("d_gbuf", gbuf),
                      ("d_tusb", tusb), ("d_tnxn", tnxn), ("d_wbf", wbf),
                      ("d_pbuf", pbuf), ("d_tubf", tubf), ("d_corr", corrs)):
            dma(ins[nm][:], t[:])
    nc.scalar.activation(acc[:], acc[:], AF.Ln)
    nc.vector.tensor_scalar(out=logits_sb[:], in0=acc[:], scalar1=-sc["CC"],
                            scalar2=bias_sb[:], op0=OP.mult, op1=OP.add)
    dma(ins["out"][:], logits_sb[:])

    for p in (psC, psN, psM, psA, spool, pool):
        p.release()


def build_program(sc):
    import concourse.mybir as mybir
    import concourse.tile as tile
    from concourse import bacc

    nc = bacc.Bacc("TRN2", target_bir_lowering=False, debug=False,
                   num_devices=NCORES)
    fp = mybir.dt.float32
    bf = mybir.dt.bfloat16
    shapes = dict(
        ebf=([B, NB * B], bf), etbf=([B, NB * B], bf),
        ltp=([B, NLT * B], bf), vb=([D, C * R], bf), mub=([D, C], bf),
        xqt=([D, QS], bf),
        ident_bf=([B, B], bf), twoi_bf=([B, B], bf), clsid_bf=([GC, B], bf),
        ident32=([B, B], fp), ones32=([B, 1], fp), onesr_bf=([B, C], bf),
        clsidT32=([B, GC], fp), maskb=([B, NG * C], fp),
        maskb_bf=([B, NG * C], bf),
    )
    ins = {name: nc.dram_tensor(name, shp, dt, kind="ExternalInput").ap()
           for name, (shp, dt) in shapes.items()}
    ins["out"] = nc.dram_tensor("out", [C, QS], fp, kind="ExternalOutput").ap()
    if DEBUG_DUMP:
        for nm, shp, dt in (("d_rsd32", [B, NG], fp), ("d_dmsb", [B, NG], fp),
                            ("d_trln", [B, 2 * NG], fp), ("d_bias", [C, 1], fp),
                            ("d_s2", [B, NG * B], bf), ("d_gbuf", [B, NG * QS], bf),
                            ("d_tusb", [C, WR], fp), ("d_tnxn", [C, 2 * QS], fp),
                            ("d_wbf", [B, NB * D], bf), ("d_pbuf", [B, NB * CR], bf),
                            ("d_tubf", [B, NB * WR], bf), ("d_corr", [C, QS], fp)):
            ins[nm] = nc.dram_tensor(nm, shp, dt, kind="ExternalOutput").ap()
    with tile.TileContext(nc) as tc:
        _emit(nc, tc, ins, sc)
    nc.compile()
    return nc


_BUILD_CACHE = {}


def kernel(**inputs) -> np.ndarray:
    from concourse import bass_utils

    shared, xqts, sc = _host_prep(inputs)
    key = tuple(sorted(sc.items()))
    if key not in _BUILD_CACHE:
        _BUILD_CACHE[key] = build_program(sc)
    nc = _BUILD_CACHE[key]

    in_maps = []
    for c in range(NCORES):
        im = {k: v for k, v in shared.items()}
        im["xqt"] = xqts[c]
        in_maps.append(im)
    res = bass_utils.run_bass_kernel_spmd(nc, in_maps, core_ids=list(range(NCORES)))
    logits = np.concatenate([r["out"].T for r in res.results], axis=0)
    return logits.astype(np.float32)


if __name__ == "__main__":
    rng = np.random.default_rng(0)
    demo = dict(
        X_support=rng.standard_normal((C * S, D), dtype=np.float32),
        y=np.repeat(np.arange(C, dtype=np.int64), S),
        X_query=rng.standard_normal((Q, D), dtype=np.float32),
        m=0.01 * rng.standard_normal((1, D)).astype(np.float32),
        kappa=np.float32(0.1), nu=np.float32(D),
        triu_diag=np.ones(D, dtype=np.float32),
        triu_lower=(np.eye(D) + 0.01 * rng.standard_normal((D, D))).astype(np.float32),
    )
    out = kernel(**demo)
    print(out.shape, out.dtype, np.abs(out).max())


# revision 18
# speedup vs baseline: 3.5286x; 1.0293x over previous
"""Trainium2 Bass kernel for nn_MetaQDA_FB (MetaQDA Fisher-Bayes logits).

Math: sigma_c = scale * (L L^T + V_c V_c^T).  The 16 centered shots are
host-projected onto a fixed orthonormal basis U of 1-perp (Y = Xg U, exact
since 1^T U = 0), giving V_c = [Y_c, sqrt(beta)(mean_c - m)] of rank R=16,
so 8 classes pack exactly into 128 partitions (NG=8 groups of GC=8).

Per-class inversion/logdet uses a shared triangular inverse W = L^{-1}
(blocked degree-3 Neumann-by-squaring on the diagonal + forward
substitution, all bf16 matmuls) plus 16x16 capacitance matrices
M_c = I + (W V_c)^T (W V_c).  M_c^{-1} is approximated on the PE with a
Jacobi-preconditioned Neumann series: J = rsd (M - D) rsd (||J|| ~ 0.4),
S = (I - J)(I + J^2) per 8-class group as dense [128,128] bf16 matmuls,
logdet M = sum ln(diag) - tr(J^2)/2.

Queries are sharded across the 8 cores (256 each); every core redundantly
builds the (cheap) per-class data and scores its own query block:

  maha_qc = (1-REG)/scale * (||W(x_q-mu_c)||^2 - g^T K_c g) + REG ||x_q-mu_c||^2
  logits  = bias_c - 0.5(common+d) * ln(common + maha)

Host does input reordering and linear prep (sort, U-projection, means, mu,
L-block extraction); all O(n^3) compute runs on device.
"""

import math
import sys

import numpy as np

for _p in ("/opt/trn_rl_repo",):
    if _p not in sys.path:
        sys.path.append(_p)

D, C, S, Q, REG, EPS = 640, 64, 16, 2048, 0.3, 1e-6
B = 128
NB = D // B            # 5 row/col blocks of L
R = 16                 # rank per class after U-projection
GC = 8                 # classes per group (GC*R = 128)
NG = C // GC           # 8 groups
NCORES = 8
QS = Q // NCORES       # queries per core
WR = QS + C            # fused [t | u] rhs width
CR = C * R             # 1024
NLT = NB * (NB - 1) // 2   # strict-upper LT block pairs
STOP_AFTER = 99        # debug: truncate kernel after phase N
DEBUG_DUMP = False     # dump intermediates as extra outputs
F32 = np.float32


def _bf16(x):
    import ml_dtypes
    return np.asarray(x, dtype=F32).astype(ml_dtypes.bfloat16)


def _lt_pairs():
    return [(k, i) for k in range(NB) for i in range(k + 1, NB)]


def _host_prep(inputs):
    Xs = np.asarray(inputs["X_support"], dtype=np.float64)
    y = np.asarray(inputs["y"])
    Xq = np.asarray(inputs["X_query"], dtype=F32)
    m = np.asarray(inputs["m"], dtype=np.float64).reshape(-1)
    kappa = float(np.asarray(inputs["kappa"]))
    nu = float(np.asarray(inputs["nu"]))
    td = np.asarray(inputs["triu_diag"], dtype=np.float64).reshape(-1)
    tl = np.asarray(inputs["triu_lower"], dtype=np.float64)

    perm = np.argsort(y, kind="stable")
    Xg = Xs[perm].reshape(C, S, D)

    mask = np.tril(np.ones((D, D)), k=-1)
    L = np.diag(np.abs(td)) + tl * mask
    LT = L.T

    kappa_ = abs(kappa) + EPS
    nu_ = max(nu, D - 1 + EPS)
    Nj = float(S)
    scale = (kappa_ + Nj + 1.0) / ((nu_ + Nj - D + 1.0) * (kappa_ + Nj))
    common = nu_ + Nj + 1.0 - D
    beta = kappa_ * Nj / (kappa_ + Nj)
    lda = float(np.sum(np.log(td ** 2)))
    BC0 = (math.lgamma(0.5 * (common + D)) - math.lgamma(0.5 * common)
           - 0.5 * D * math.log(common)
           - 0.5 * D * math.log(scale)
           - 0.5 * lda
           + 0.5 * (common + D) * math.log(common))
    sc = dict(
        common=common, ca=(1.0 - REG) / scale,
        BC0=BC0, CC=0.5 * (common + D),
    )

    # U: orthonormal basis of 1-perp in R^S  (fixed, exact to fp32)
    Ac = np.eye(S) - np.ones((S, S)) / S
    Uq, _ = np.linalg.qr(Ac)
    U15 = Uq[:, :S - 1]                                       # [16, 15]

    mean = Xg.mean(axis=1)                                    # [C, D]
    mu = (kappa_ / (kappa_ + Nj)) * m + (Nj / (kappa_ + Nj)) * mean
    XgU = np.einsum('csd,st->cdt', Xg, U15)                   # [C, D, 15]
    v_host = np.zeros((D, C * R), dtype=np.float64)           # [D, (c r)]
    for c in range(C):
        v_host[:, c * R:c * R + (S - 1)] = XgU[c]
        v_host[:, c * R + (S - 1)] = math.sqrt(beta) * (mean[c] - m)

    # E blocks (negated strict lower of diag blocks), T1 = I + E
    ebf = np.zeros((B, NB * B))
    etbf = np.zeros((B, NB * B))
    ltp = np.zeros((B, NLT * B))
    for b in range(NB):
        Lbb = L[b * B:(b + 1) * B, b * B:(b + 1) * B]
        E = -np.tril(Lbb, -1)
        ebf[:, b * B:(b + 1) * B] = E
        etbf[:, b * B:(b + 1) * B] = E.T
    for j, (k, i) in enumerate(_lt_pairs()):
        ltp[:, j * B:(j + 1) * B] = LT[k * B:(k + 1) * B, i * B:(i + 1) * B]

    # constants
    ident_bf = np.eye(B)
    twoi_bf = 2.0 * np.eye(B)
    ident32 = np.eye(B, dtype=F32)
    ones32 = np.ones((B, 1), dtype=F32)
    onesr_bf = np.ones((B, C))
    clsid_bf = np.zeros((GC, B))          # [q, p] = 1 iff p//R == q
    clsidT32 = np.zeros((B, GC), dtype=F32)
    for p in range(B):
        clsid_bf[p // R, p] = 1.0
        clsidT32[p, p // R] = 1.0
    maskb = np.zeros((B, NG * C), dtype=F32)
    for g in range(NG):
        for p in range(B):
            maskb[p, g * C + g * GC + p // R] = 1.0

    shared = dict(
        ebf=_bf16(ebf), etbf=_bf16(etbf), ltp=_bf16(ltp),
        vb=_bf16(v_host), mub=_bf16(mu.T),
        ident_bf=_bf16(ident_bf), twoi_bf=_bf16(twoi_bf),
        clsid_bf=_bf16(clsid_bf),
        ident32=ident32, ones32=ones32, onesr_bf=_bf16(onesr_bf),
        clsidT32=clsidT32, maskb=maskb, maskb_bf=_bf16(maskb),
    )
    xqts = [_bf16(np.ascontiguousarray(Xq[c * QS:(c + 1) * QS].T))
            for c in range(NCORES)]
    return shared, xqts, sc


def _emit(nc, tc, ins, sc):
    import concourse.mybir as mybir

    fp = mybir.dt.float32
    bf = mybir.dt.bfloat16
    fpr = mybir.dt.float32r
    AF = mybir.ActivationFunctionType
    OP = mybir.AluOpType
    AX = mybir.AxisListType

    pool = tc.alloc_tile_pool(name="persist", bufs=1)
    spool = tc.alloc_tile_pool(name="scratch", bufs=2)
    psA = tc.alloc_tile_pool(name="psA", bufs=2, space="PSUM")   # big stream
    psM = tc.alloc_tile_pool(name="psM", bufs=2, space="PSUM")   # M groups
    psN = tc.alloc_tile_pool(name="psN", bufs=3, space="PSUM")   # NK smalls
    psC = tc.alloc_tile_pool(name="psC", bufs=1, space="PSUM")   # corr accum

    mm = nc.tensor.matmul

    def mmr(out, lhsT, rhs, **kw):
        mm(out, lhsT.bitcast(fpr), rhs.bitcast(fpr), **kw)

    dma = nc.sync.dma_start
    _dma_engines = [nc.sync, nc.gpsimd, nc.scalar]
    _dma_rr = [0]

    def dma_rr(out, in_):
        eng = _dma_engines[_dma_rr[0] % len(_dma_engines)]
        _dma_rr[0] += 1
        eng.dma_start(out, in_)

    # psum -> sbuf cast/copy round-robin (DVE / Act only: both read PSUM)
    _cast_rr = [0]

    def cast(out, in_):
        if _cast_rr[0] % 2 == 0:
            nc.vector.tensor_copy(out, in_)
        else:
            nc.scalar.copy(out, in_)
        _cast_rr[0] += 1

    # ---- persistent SBUF tensors ----
    def T(name, shape, dt=fp):
        return pool.tile(shape, dt, name=name, tag=name)

    ebf = T("ebf", [B, NB * B], bf)
    etbf = T("etbf", [B, NB * B], bf)
    t1bf = T("t1bf", [B, NB * B], bf)
    t1tbf = T("t1tbf", [B, NB * B], bf)
    ltp = T("ltp", [B, NLT * B], bf)
    wbf = T("wbf", [B, NB * D], bf)        # W block (i,j) at i*D+j*B
    wtbf = T("wtbf", [B, NB * D], bf)      # W^T block (a,b) at a*D+b*B
    vbuf = T("vbuf", [B, NB * CR], bf)
    xmu = T("xmu", [B, NB * WR], bf)       # [xq | mu] per k block
    tubf = T("tubf", [B, NB * WR], bf)     # [t | u] per i block
    t2x2 = T("t2x2", [B, NB * 2 * QS], bf)
    pbuf = T("pbuf", [B, NB * CR], bf)
    s2sb = T("s2sb", [B, NG * B], bf)      # per-group Neumann core S
    gbuf = T("gbuf", [B, NG * QS], bf)
    rsd32 = T("rsd32", [B, NG])
    dmsb = T("dmsb", [B, NG])
    trln = T("trln", [B, 2 * NG])          # [tr2 cols | lnD cols]
    tusb = T("tusb", [C, WR])
    xmusb = T("xmusb", [C, WR])
    tnxn = T("tnxn", [C, 2 * QS])
    un_sb = T("un_sb", [C, 1])
    mun_sb = T("mun_sb", [C, 1])
    bias_sb = T("bias_sb", [C, 1])
    logits_sb = T("logits_sb", [C, QS])
    scr64 = T("scr64", [C, C])
    # consts
    ident_bf = T("ident_bf", [B, B], bf)
    twoi_bf = T("twoi_bf", [B, B], bf)
    clsid_bf = T("clsid_bf", [GC, B], bf)
    ident32 = T("ident32", [B, B])
    ones32 = T("ones32", [B, 1])
    onesr_bf = T("onesr_bf", [B, C], bf)
    clsidT32 = T("clsidT32", [B, GC])
    maskb = T("maskb", [B, NG * C])
    maskb_bf = T("maskb_bf", [B, NG * C], bf)

    # ---- input DMAs (W-phase inputs first; spread queues) ----
    dma(ident_bf[:], ins["ident_bf"][:])
    dma(ebf[:], ins["ebf"][:])
    nc.gpsimd.dma_start(etbf[:], ins["etbf"][:])
    nc.scalar.dma_start(ltp[:], ins["ltp"][:])
    nc.gpsimd.dma_start(vbuf.rearrange("p (b n) -> p b n", b=NB),
                        ins["vb"].rearrange("(b p) n -> p b n", p=B))
    dma(xmu.rearrange("p (b w) -> p b w", w=WR)[:, :, 0:QS],
        ins["xqt"].rearrange("(b p) n -> p b n", p=B))
    nc.scalar.dma_start(xmu.rearrange("p (b w) -> p b w", w=WR)[:, :, QS:],
                        ins["mub"].rearrange("(b p) c -> p b c", p=B))
    for cname, ct in (("twoi_bf", twoi_bf), ("clsid_bf", clsid_bf),
                      ("ident32", ident32), ("ones32", ones32),
                      ("onesr_bf", onesr_bf), ("clsidT32", clsidT32),
                      ("maskb", maskb), ("maskb_bf", maskb_bf)):
        dma_rr(ct[:], ins[cname][:])

    # zero the strictly-upper W blocks (read as zeros in fwd substitution)
    for k in range(NB - 1):
        nc.gpsimd.memset(wbf[:, k * D + (k + 1) * B: (k + 1) * D], 0.0)

    # T1 = I + E (broadcast identity across the 5 blocks)
    ib = ident_bf[:, None, :].broadcast_to([B, NB, B])
    nc.vector.tensor_add(t1bf.rearrange("p (b n) -> p b n", b=NB),
                         ebf.rearrange("p (b n) -> p b n", b=NB), ib)
    nc.vector.tensor_add(t1tbf.rearrange("p (b n) -> p b n", b=NB),
                         etbf.rearrange("p (b n) -> p b n", b=NB), ib)

    def _gate(n):
        if STOP_AFTER <= n:
            nc.vector.memset(logits_sb[:], 0.0)
            dma(ins["out"][:], logits_sb[:])
            for p in (psC, psN, psM, psA, spool, pool):
                p.release()
            return True
        return False

    eb = lambda b: ebf[:, b * B:(b + 1) * B]
    etb = lambda b: etbf[:, b * B:(b + 1) * B]
    t1b = lambda b: t1bf[:, b * B:(b + 1) * B]
    t1tb = lambda b: t1tbf[:, b * B:(b + 1) * B]
    w_blk = lambda i, j: wbf[:, i * D + j * B: i * D + (j + 1) * B]
    wt_blk = lambda a, b: wtbf[:, a * D + b * B: a * D + (b + 1) * B]
    _ltidx = {ki: j for j, ki in enumerate(_lt_pairs())}
    lt_blk = lambda k, i: ltp[:, _ltidx[(k, i)] * B:(_ltidx[(k, i)] + 1) * B]

    # =========== phase W-diag: W_bb = (I+E)(I+E^2), deg-3 Neumann ===========
    for b in range(NB):
        e2tps = psN.tile([B, B], fp, name=f"e2tps{b}", tag="nks")
        mm(e2tps[:], eb(b), etb(b), start=True, stop=True)
        e2tbf = spool.tile([B, B], bf, name=f"e2tbf{b}", tag="e2tbf", bufs=3)
        cast(e2tbf[:], e2tps[:])
        wps = psN.tile([B, B], fp, name=f"wps{b}", tag="nks")
        mm(wps[:], e2tbf[:], t1b(b), start=True, stop=False)
        mm(wps[:], ident_bf[:], t1b(b), start=False, stop=True)
        cast(w_blk(b, b), wps[:])
        wtps = psN.tile([B, B], fp, name=f"wtps{b}", tag="nks")
        mm(wtps[:], t1b(b), e2tbf[:], start=True, stop=False)
        mm(wtps[:], ident_bf[:], t1tb(b), start=False, stop=True)
        cast(wt_blk(b, b), wtps[:])

    # preload the Sqrt activation table off the critical path
    akscr = spool.tile([1, 1], fp, name="akscr", tag="akscr", bufs=1)
    nc.scalar.activation(akscr[:], ones32[0:1, 0:1], AF.Sqrt)

    if _gate(1):
        return
    # =========== phase W-offdiag: row-batched forward substitution ===========
    for i in range(1, NB):
        accps = psA.tile([B, i * B], fp, name=f"acc{i}", tag="bigA")
        for k in range(i):
            mm(accps[:], lt_blk(k, i), wbf[:, k * D: k * D + i * B],
               start=(k == 0), stop=(k == i - 1))
        tij = spool.tile([B, i * B], bf, name=f"tij{i}", tag="tij", bufs=3)
        cast(tij[:], accps[:])
        wps2 = psA.tile([B, i * B], fp, name=f"wo{i}", tag="bigA")
        mm(wps2[:], wt_blk(i, i), tij[:], start=True, stop=True)
        nc.vector.tensor_scalar(out=wbf[:, i * D: i * D + i * B], in0=wps2[:],
                                scalar1=-1.0, scalar2=None, op0=OP.mult)
        for j in range(i):
            trps = psN.tile([B, B], bf, name=f"tr{i}{j}", tag="nks")
            nc.tensor.transpose(trps[:], w_blk(i, j), ident_bf[:])
            cast(wt_blk(j, i), trps[:])

    if _gate(2):
        return
    # =========== P = W @ V (bf16) ===========
    for i in range(NB):
        for ch in range(2):
            pps = psA.tile([B, CR // 2], fp, name=f"p{i}{ch}", tag="bigA")
            for k in range(i + 1):
                mm(pps[:], wt_blk(k, i),
                   vbuf[:, k * CR + ch * (CR // 2): k * CR + (ch + 1) * (CR // 2)],
                   start=(k == 0), stop=(k == i))
            cast(pbuf[:, i * CR + ch * (CR // 2): i * CR + (ch + 1) * (CR // 2)],
                 pps[:])

    if _gate(3):
        return
    # =========== t = W xq, u = W mu (fused rhs = [xq | mu]) ===========
    for i in range(NB):
        tups = psA.tile([B, WR], fp, name=f"tu{i}", tag="bigA")
        for k in range(i + 1):
            mm(tups[:], wt_blk(k, i), xmu[:, k * WR:(k + 1) * WR],
               start=(k == 0), stop=(k == i))
        nc.vector.tensor_copy(tubf[:, i * WR:(i + 1) * WR], tups[:])
        nc.scalar.square(t2x2[:, i * 2 * QS: i * 2 * QS + QS], tups[:, 0:QS])
        nc.scalar.square(t2x2[:, i * 2 * QS + QS:(i + 1) * 2 * QS],
                         xmu[:, i * WR: i * WR + QS])

    # =========== tu = u^T [t|u], xmu = mu^T [xq|mu] ===========
    ptu = psA.tile([C, WR], fp, name="ptu", tag="bigA")
    for k in range(NB):
        mm(ptu[:], tubf[:, k * WR + QS:(k + 1) * WR],
           tubf[:, k * WR:(k + 1) * WR], start=(k == 0), stop=(k == NB - 1))
    nc.scalar.copy(tusb[:], ptu[:])
    pxmu = psA.tile([C, WR], fp, name="pxmu", tag="bigA")
    for k in range(NB):
        mm(pxmu[:], xmu[:, k * WR + QS:(k + 1) * WR],
           xmu[:, k * WR:(k + 1) * WR], start=(k == 0), stop=(k == NB - 1))
    nc.scalar.copy(xmusb[:], pxmu[:])
    nc.vector.tensor_mul(scr64[:], tusb[:, QS:], ident32[0:C, 0:C])
    nc.vector.tensor_reduce(un_sb[:], scr64[:], AX.X, OP.add)
    nc.vector.tensor_mul(scr64[:], xmusb[:, QS:], ident32[0:C, 0:C])
    nc.vector.tensor_reduce(mun_sb[:], scr64[:], AX.X, OP.add)

    # =========== tn | xn row sums (replicated over classes) ===========
    ptn = psA.tile([C, 2 * QS], fp, name="ptn", tag="bigA")
    for bk in range(NB):
        mm(ptn[:], onesr_bf[:], t2x2[:, bk * 2 * QS:(bk + 1) * 2 * QS],
           start=(bk == 0), stop=(bk == NB - 1))
    nc.scalar.copy(tnxn[:], ptn[:])

    if _gate(4):
        return
    # =========== M_g = P_g^T P_g  (two [B,512] psum tiles, 4 groups each) ===========
    mts = [psM.tile([B, 4 * B], fp, name=f"mts{h}", tag="psM") for h in range(2)]
    mreg = lambda g: mts[g // 4][:, (g % 4) * B:(g % 4 + 1) * B]
    for g in range(NG):
        pslc = lambda k: pbuf[:, k * CR + g * B: k * CR + (g + 1) * B]
        for k in range(NB):
            mm(mreg(g), pslc(k), pslc(k), start=(k == 0), stop=False)
        mm(mreg(g), ident_bf[:], ident_bf[:], start=False, stop=True)  # M = I + P^T P

    # =========== NK feeds: diag, rsd, masked col-scale, J, I-J ===========
    mi_l, jbf_l, imj_l, msb_l = [], [], [], []
    for g in range(NG):
        msb = spool.tile([B, B], fp, name=f"msb{g}", tag="msb", bufs=NG)
        nc.scalar.copy(msb[:], mreg(g))
        msb_l.append(msb)
    for g in range(NG):
        mi = spool.tile([B, B], fp, name=f"mi{g}", tag="mi", bufs=NG)
        nc.gpsimd.tensor_mul(mi[:], msb_l[g][:], ident32[:])
        mi_l.append(mi)
    dm_l = []
    for g in range(NG):
        dmps = psN.tile([B, 1], fp, name=f"dm{g}", tag="nks")
        mm(dmps[:], mi_l[g][:], ones32[:], start=True, stop=True)
        dm_l.append(dmps)
    rsdbf_l = []
    for g in range(NG):
        nc.vector.tensor_copy(dmsb[:, g:g + 1], dm_l[g][:])
        rcp = spool.tile([B, 1], fp, name=f"rcp{g}", tag="rcp", bufs=NG)
        nc.vector.reciprocal(rcp[:], dm_l[g][:])
        rsdbf = spool.tile([B, 1], bf, name=f"rsdb{g}", tag="rsdb", bufs=NG)
        nc.scalar.activation(rsdbf[:], rcp[:], AF.Sqrt)
        rsdbf_l.append(rsdbf)
    spr_l = []
    for g in range(NG):
        nc.vector.tensor_copy(rsd32[:, g:g + 1], rsdbf_l[g][:])
        spr = spool.tile([B, GC], bf, name=f"spr{g}", tag="spr", bufs=NG)
        nc.vector.tensor_mul(
            spr[:], rsd32[:, g:g + 1].broadcast_to([B, GC]), clsidT32[:])
        spr_l.append(spr)
    sps_l = []
    for g in range(NG):
        spps = psN.tile([GC, B], bf, name=f"spp{g}", tag="nks")
        nc.tensor.transpose(spps[:], spr_l[g][:], ident_bf[:])
        spsb = spool.tile([GC, B], bf, name=f"sps{g}", tag="sps", bufs=NG)
        nc.scalar.copy(spsb[:], spps[:])
        sps_l.append(spsb)
    rmf_l = []
    for g in range(NG):
        rmfps = psN.tile([B, B], fp, name=f"rmf{g}", tag="nks")
        mm(rmfps[:], clsid_bf[:], sps_l[g][:], start=True, stop=True)
        rmf = spool.tile([B, B], fp, name=f"rmfs{g}", tag="rmfs", bufs=NG)
        nc.scalar.copy(rmf[:], rmfps[:])
        rmf_l.append(rmf)
    for g in range(NG):
        jraw = spool.tile([B, B], bf, name=f"jraw{g}", tag="jraw", bufs=NG)
        nc.vector.scalar_tensor_tensor(
            out=jraw[:], in0=msb_l[g][:], scalar=rsd32[:, g:g + 1], in1=rmf_l[g][:],
            op0=OP.mult, op1=OP.mult)
        jbf = spool.tile([B, B], bf, name=f"jbf{g}", tag="jbf", bufs=NG)
        nc.vector.tensor_sub(jbf[:], jraw[:], ident_bf[:])
        jbf_l.append(jbf)
        imj = spool.tile([B, B], bf, name=f"imj{g}", tag="imj", bufs=NG)
        nc.gpsimd.tensor_sub(imj[:], twoi_bf[:], jraw[:])
        imj_l.append(imj)

    if _gate(5):
        return
    # =========== g = P_g^T [t|u] - b, scaled by rsd ===========
    for g in range(NG):
        pg = psA.tile([B, WR], fp, name=f"pg{g}", tag="bigA")
        for k in range(NB):
            mm(pg[:], pbuf[:, k * CR + g * B: k * CR + (g + 1) * B],
               tubf[:, k * WR:(k + 1) * WR], start=(k == 0), stop=(k == NB - 1))
        bscr = spool.tile([B, C], fp, name=f"bscr{g}", tag="bscr", bufs=4)
        nc.vector.tensor_mul(bscr[:], pg[:, QS:], maskb[:, g * C:(g + 1) * C])
        bg = spool.tile([B, 1], fp, name=f"bg{g}", tag="bg", bufs=4)
        nc.vector.tensor_reduce(bg[:], bscr[:], AX.X, OP.add)
        nc.vector.tensor_scalar(out=gbuf[:, g * QS:(g + 1) * QS],
                                in0=pg[:, 0:QS], scalar1=bg[:],
                                scalar2=rsd32[:, g:g + 1],
                                op0=OP.subtract, op1=OP.mult)

    if _gate(6):
        return
    # =========== NK matmuls: S_g = (I + J^2)(I - J) ===========
    for g in range(NG):
        j2ps = psN.tile([B, B], fp, name=f"j2{g}", tag="nks")
        mm(j2ps[:], jbf_l[g][:], jbf_l[g][:], start=True, stop=True)
        j2bf = spool.tile([B, B], bf, name=f"j2b{g}", tag="j2b", bufs=NG)
        cast(j2bf[:], j2ps[:])
        s1ps = psN.tile([B, B], fp, name=f"s1{g}", tag="nks")
        mm(s1ps[:], j2bf[:], imj_l[g][:], start=True, stop=False)
        mm(s1ps[:], ident_bf[:], imj_l[g][:], start=False, stop=True)
        cast(s2sb[:, g * B:(g + 1) * B], s1ps[:])
        sq = spool.tile([B, B], fp, name=f"sq{g}", tag="sq", bufs=4)
        nc.scalar.square(sq[:], jbf_l[g][:])
        t2g = psN.tile([B, 1], fp, name=f"t2g{g}", tag="nks")
        mm(t2g[:], sq[:], ones32[:], start=True, stop=True)
        nc.vector.tensor_copy(trln[:, g:g + 1], t2g[:])

    # lnD after all Rsqrt ops (one act-table switch)
    nc.scalar.activation(trln[:, NG:], dmsb[:], AF.Ln)

    if _gate(7):
        return
    # =========== Kh, corr (two accumulators), class-summed bias ===========
    corrA = psC.tile([C, QS], fp, name="corrA", tag="corrA")
    corrB = psM.tile([C, QS], fp, name="corrB", tag="psM")
    for g in range(NG):
        hps = psA.tile([B, QS], fp, name=f"h{g}", tag="bigA")
        mm(hps[:], s2sb[:, g * B:(g + 1) * B], gbuf[:, g * QS:(g + 1) * QS],
           start=True, stop=True)
        prod = spool.tile([B, QS], bf, name=f"prod{g}", tag="prod", bufs=NG)
        if g % 2 == 0:
            nc.vector.tensor_mul(prod[:], hps[:], gbuf[:, g * QS:(g + 1) * QS])
        else:
            hsb = spool.tile([B, QS], fp, name=f"hsb{g}", tag="hsb", bufs=4)
            nc.scalar.copy(hsb[:], hps[:])
            nc.gpsimd.tensor_mul(prod[:], hsb[:], gbuf[:, g * QS:(g + 1) * QS])
        tgt = corrA if g % 2 == 0 else corrB
        mm(tgt[:], maskb_bf[:, g * C:(g + 1) * C], prod[:],
           start=(g < 2), stop=(g >= NG - 2))

    clsps = psN.tile([C, 2], fp, name="clsps", tag="nks")
    trv = trln.rearrange("p (two g) -> p g two", g=NG)
    for g in range(NG):
        mm(clsps[:], maskb[:, g * C:(g + 1) * C], trv[:, g, :],
           start=(g == 0), stop=(g == NG - 1))
    nc.vector.tensor_scalar(out=bias_sb[:], in0=clsps[:, 1:2], scalar1=-0.5,
                            scalar2=sc["BC0"], op0=OP.mult, op1=OP.add)
    nc.vector.scalar_tensor_tensor(out=bias_sb[:], in0=clsps[:, 0:1],
                                   scalar=0.25, in1=bias_sb[:],
                                   op0=OP.mult, op1=OP.add)

    if _gate(8):
        return
    # =========== assemble logits ===========
    wda = spool.tile([C, QS], fp, name="wda", tag="wda", bufs=1)
    d2a = spool.tile([C, QS], fp, name="d2a", tag="d2a", bufs=1)
    acc = spool.tile([C, QS], fp, name="acc", tag="acc", bufs=1)
    # wd2 = tn - 2*tu + un
    nc.vector.scalar_tensor_tensor(out=wda[:], in0=tusb[:, 0:QS], scalar=-2.0,
                                   in1=tnxn[:, 0:QS], op0=OP.mult, op1=OP.add)
    nc.vector.tensor_scalar(out=wda[:], in0=wda[:], scalar1=un_sb[:],
                            scalar2=None, op0=OP.add)
    # d2 + mun + common/REG
    nc.vector.scalar_tensor_tensor(out=d2a[:], in0=xmusb[:, 0:QS], scalar=-2.0,
                                   in1=tnxn[:, QS:], op0=OP.mult, op1=OP.add)
    nc.vector.tensor_scalar(out=d2a[:], in0=d2a[:], scalar1=mun_sb[:],
                            scalar2=sc["common"] / REG, op0=OP.add, op1=OP.add)
    # acc = ca*(wd2 - corrA - corrB) + REG*d2' = maha + common
    nc.vector.tensor_sub(acc[:], wda[:], corrA[:])
    nc.vector.tensor_sub(acc[:], acc[:], corrB[:])
    nc.vector.tensor_scalar(out=acc[:], in0=acc[:], scalar1=sc["ca"],
                            scalar2=None, op0=OP.mult)
    nc.vector.scalar_tensor_tensor(out=acc[:], in0=d2a[:], scalar=REG,
                                   in1=acc[:], op0=OP.mult, op1=OP.add)
    if DEBUG_DUMP:
        corrs = spool.tile([C, QS], fp, name="corrs", tag="corrs", bufs=1)
        nc.vector.tensor_add(corrs[:], corrA[:], corrB[:])
        for nm, t in (("d_rsd32", rsd32), ("d_dmsb", dmsb), ("d_trln", trln),
                      ("d_bias", bias_sb), ("d_s2", s2sb), ("d_gbuf", gbuf),
                      ("d_tusb", tusb), ("d_tnxn", tnxn), ("d_wbf", wbf),
                      ("d_pbuf", pbuf), ("d_tubf", tubf), ("d_corr", corrs)):
            dma(ins[nm][:], t[:])
    nc.scalar.activation(acc[:], acc[:], AF.Ln)
    nc.vector.tensor_scalar(out=logits_sb[:], in0=acc[:], scalar1=-sc["CC"],
                            scalar2=bias_sb[:], op0=OP.mult, op1=OP.add)
    dma(ins["out"][:], logits_sb[:])

    for p in (psC, psN, psM, psA, spool, pool):
        p.release()


def build_program(sc):
    import concourse.mybir as mybir
    import concourse.tile as tile
    from concourse import bacc

    nc = bacc.Bacc("TRN2", target_bir_lowering=False, debug=False,
                   num_devices=NCORES)
    fp = mybir.dt.float32
    bf = mybir.dt.bfloat16
    shapes = dict(
        ebf=([B, NB * B], bf), etbf=([B, NB * B], bf),
        ltp=([B, NLT * B], bf), vb=([D, C * R], bf), mub=([D, C], bf),
        xqt=([D, QS], bf),
        ident_bf=([B, B], bf), twoi_bf=([B, B], bf), clsid_bf=([GC, B], bf),
        ident32=([B, B], fp), ones32=([B, 1], fp), onesr_bf=([B, C], bf),
        clsidT32=([B, GC], fp), maskb=([B, NG * C], fp),
        maskb_bf=([B, NG * C], bf),
    )
    ins = {name: nc.dram_tensor(name, shp, dt, kind="ExternalInput").ap()
           for name, (shp, dt) in shapes.items()}
    ins["out"] = nc.dram_tensor("out", [C, QS], fp, kind="ExternalOutput").ap()
    if DEBUG_DUMP:
        for nm, shp, dt in (("d_rsd32", [B, NG], fp), ("d_dmsb", [B, NG], fp),
                            ("d_trln", [B, 2 * NG], fp), ("d_bias", [C, 1], fp),
                            ("d_s2", [B, NG * B], bf), ("d_gbuf", [B, NG * QS], bf),
                            ("d_tusb", [C, WR], fp), ("d_tnxn", [C, 2 * QS], fp),
                            ("d_wbf", [B, NB * D], bf), ("d_pbuf", [B, NB * CR], bf),
                            ("d_tubf", [B, NB * WR], bf), ("d_corr", [C, QS], fp)):
            ins[nm] = nc.dram_tensor(nm, shp, dt, kind="ExternalOutput").ap()
    with tile.TileContext(nc) as tc:
        _emit(nc, tc, ins, sc)
    nc.compile()
    return nc


_BUILD_CACHE = {}


def kernel(**inputs) -> np.ndarray:
    from concourse import bass_utils

    shared, xqts, sc = _host_prep(inputs)
    key = tuple(sorted(sc.items()))
    if key not in _BUILD_CACHE:
        _BUILD_CACHE[key] = build_program(sc)
    nc = _BUILD_CACHE[key]

    in_maps = []
    for c in range(NCORES):
        im = {k: v for k, v in shared.items()}
        im["xqt"] = xqts[c]
        in_maps.append(im)
    res = bass_utils.run_bass_kernel_spmd(nc, in_maps, core_ids=list(range(NCORES)))
    logits = np.concatenate([r["out"].T for r in res.results], axis=0)
    return logits.astype(np.float32)


if __name__ == "__main__":
    rng = np.random.default_rng(0)
    demo = dict(
        X_support=rng.standard_normal((C * S, D), dtype=np.float32),
        y=np.repeat(np.arange(C, dtype=np.int64), S),
        X_query=rng.standard_normal((Q, D), dtype=np.float32),
        m=0.01 * rng.standard_normal((1, D)).astype(np.float32),
        kappa=np.float32(0.1), nu=np.float32(D),
        triu_diag=np.ones(D, dtype=np.float32),
        triu_lower=(np.eye(D) + 0.01 * rng.standard_normal((D, D))).astype(np.float32),
    )
    out = kernel(**demo)
    print(out.shape, out.dtype, np.abs(out).max())


# revision 19
# speedup vs baseline: 3.5444x; 1.0045x over previous
"""Trainium2 Bass kernel for nn_MetaQDA_FB (MetaQDA Fisher-Bayes logits).

Math: sigma_c = scale * (L L^T + V_c V_c^T).  The 16 centered shots are
host-projected onto a fixed orthonormal basis U of 1-perp (Y = Xg U, exact
since 1^T U = 0), giving V_c = [Y_c, sqrt(beta)(mean_c - m)] of rank R=16,
so 8 classes pack exactly into 128 partitions (NG=8 groups of GC=8).

Per-class inversion/logdet uses a shared triangular inverse W = L^{-1}
(blocked degree-3 Neumann-by-squaring on the diagonal + forward
substitution, all bf16 matmuls) plus 16x16 capacitance matrices
M_c = I + (W V_c)^T (W V_c).  M_c^{-1} is approximated on the PE with a
Jacobi-preconditioned Neumann series: J = rsd (M - D) rsd (||J|| ~ 0.4),
S = (I - J)(I + J^2) per 8-class group as dense [128,128] bf16 matmuls,
logdet M = sum ln(diag) - tr(J^2)/2.

Queries are sharded across the 8 cores (256 each); every core redundantly
builds the (cheap) per-class data and scores its own query block:

  maha_qc = (1-REG)/scale * (||W(x_q-mu_c)||^2 - g^T K_c g) + REG ||x_q-mu_c||^2
  logits  = bias_c - 0.5(common+d) * ln(common + maha)

Host does input reordering and linear prep (sort, U-projection, means, mu,
L-block extraction); all O(n^3) compute runs on device.
"""

import math
import sys

import numpy as np

for _p in ("/opt/trn_rl_repo",):
    if _p not in sys.path:
        sys.path.append(_p)

D, C, S, Q, REG, EPS = 640, 64, 16, 2048, 0.3, 1e-6
B = 128
NB = D // B            # 5 row/col blocks of L
R = 16                 # rank per class after U-projection
GC = 8                 # classes per group (GC*R = 128)
NG = C // GC           # 8 groups
NCORES = 8
QS = Q // NCORES       # queries per core
WR = QS + C            # fused [t | u] rhs width
CR = C * R             # 1024
NLT = NB * (NB - 1) // 2   # strict-upper LT block pairs
STOP_AFTER = 99        # debug: truncate kernel after phase N
DEBUG_DUMP = False     # dump intermediates as extra outputs
F32 = np.float32


def _bf16(x):
    import ml_dtypes
    return np.asarray(x, dtype=F32).astype(ml_dtypes.bfloat16)


def _lt_pairs():
    return [(k, i) for k in range(NB) for i in range(k + 1, NB)]


def _host_prep(inputs):
    Xs = np.asarray(inputs["X_support"], dtype=np.float64)
    y = np.asarray(inputs["y"])
    Xq = np.asarray(inputs["X_query"], dtype=F32)
    m = np.asarray(inputs["m"], dtype=np.float64).reshape(-1)
    kappa = float(np.asarray(inputs["kappa"]))
    nu = float(np.asarray(inputs["nu"]))
    td = np.asarray(inputs["triu_diag"], dtype=np.float64).reshape(-1)
    tl = np.asarray(inputs["triu_lower"], dtype=np.float64)

    perm = np.argsort(y, kind="stable")
    Xg = Xs[perm].reshape(C, S, D)

    mask = np.tril(np.ones((D, D)), k=-1)
    L = np.diag(np.abs(td)) + tl * mask
    LT = L.T

    kappa_ = abs(kappa) + EPS
    nu_ = max(nu, D - 1 + EPS)
    Nj = float(S)
    scale = (kappa_ + Nj + 1.0) / ((nu_ + Nj - D + 1.0) * (kappa_ + Nj))
    common = nu_ + Nj + 1.0 - D
    beta = kappa_ * Nj / (kappa_ + Nj)
    lda = float(np.sum(np.log(td ** 2)))
    BC0 = (math.lgamma(0.5 * (common + D)) - math.lgamma(0.5 * common)
           - 0.5 * D * math.log(common)
           - 0.5 * D * math.log(scale)
           - 0.5 * lda
           + 0.5 * (common + D) * math.log(common))
    sc = dict(
        common=common, ca=(1.0 - REG) / scale,
        BC0=BC0, CC=0.5 * (common + D),
    )

    # U: orthonormal basis of 1-perp in R^S  (fixed, exact to fp32)
    Ac = np.eye(S) - np.ones((S, S)) / S
    Uq, _ = np.linalg.qr(Ac)
    U15 = Uq[:, :S - 1]                                       # [16, 15]

    mean = Xg.mean(axis=1)                                    # [C, D]
    mu = (kappa_ / (kappa_ + Nj)) * m + (Nj / (kappa_ + Nj)) * mean
    XgU = np.einsum('csd,st->cdt', Xg, U15)                   # [C, D, 15]
    v_host = np.zeros((D, C * R), dtype=np.float64)           # [D, (c r)]
    for c in range(C):
        v_host[:, c * R:c * R + (S - 1)] = XgU[c]
        v_host[:, c * R + (S - 1)] = math.sqrt(beta) * (mean[c] - m)

    # E blocks (negated strict lower of diag blocks), T1 = I + E
    ebf = np.zeros((B, NB * B))
    etbf = np.zeros((B, NB * B))
    ltp = np.zeros((B, NLT * B))
    for b in range(NB):
        Lbb = L[b * B:(b + 1) * B, b * B:(b + 1) * B]
        E = -np.tril(Lbb, -1)
        ebf[:, b * B:(b + 1) * B] = E
        etbf[:, b * B:(b + 1) * B] = E.T
    for j, (k, i) in enumerate(_lt_pairs()):
        ltp[:, j * B:(j + 1) * B] = LT[k * B:(k + 1) * B, i * B:(i + 1) * B]

    # constants
    ident_bf = np.eye(B)
    twoi_bf = 2.0 * np.eye(B)
    ident32 = np.eye(B, dtype=F32)
    ones32 = np.ones((B, 1), dtype=F32)
    onesr_bf = np.ones((B, C))
    clsid_bf = np.zeros((GC, B))          # [q, p] = 1 iff p//R == q
    clsidT32 = np.zeros((B, GC), dtype=F32)
    for p in range(B):
        clsid_bf[p // R, p] = 1.0
        clsidT32[p, p // R] = 1.0
    maskb = np.zeros((B, NG * C), dtype=F32)
    for g in range(NG):
        for p in range(B):
            maskb[p, g * C + g * GC + p // R] = 1.0
    blkmask = np.zeros((B, B))
    for p in range(B):
        blkmask[p, (p // R) * R:(p // R + 1) * R] = 1.0

    shared = dict(
        ebf=_bf16(ebf), etbf=_bf16(etbf), ltp=_bf16(ltp),
        vb=_bf16(v_host), mub=_bf16(mu.T),
        ident_bf=_bf16(ident_bf), twoi_bf=_bf16(twoi_bf),
        clsid_bf=_bf16(clsid_bf),
        ident32=ident32, ones32=ones32, onesr_bf=_bf16(onesr_bf),
        clsidT32=clsidT32, maskb=maskb, maskb_bf=_bf16(maskb),
        blkmask_bf=_bf16(blkmask),
    )
    xqts = [_bf16(np.ascontiguousarray(Xq[c * QS:(c + 1) * QS].T))
            for c in range(NCORES)]
    return shared, xqts, sc


def _emit(nc, tc, ins, sc):
    import concourse.mybir as mybir

    fp = mybir.dt.float32
    bf = mybir.dt.bfloat16
    fpr = mybir.dt.float32r
    AF = mybir.ActivationFunctionType
    OP = mybir.AluOpType
    AX = mybir.AxisListType

    pool = tc.alloc_tile_pool(name="persist", bufs=1)
    spool = tc.alloc_tile_pool(name="scratch", bufs=2)
    psA = tc.alloc_tile_pool(name="psA", bufs=2, space="PSUM")   # big stream
    psM = tc.alloc_tile_pool(name="psM", bufs=2, space="PSUM")   # M groups
    psN = tc.alloc_tile_pool(name="psN", bufs=3, space="PSUM")   # NK smalls
    psC = tc.alloc_tile_pool(name="psC", bufs=1, space="PSUM")   # corr accum

    mm = nc.tensor.matmul

    def mmr(out, lhsT, rhs, **kw):
        mm(out, lhsT.bitcast(fpr), rhs.bitcast(fpr), **kw)

    dma = nc.sync.dma_start
    _dma_engines = [nc.sync, nc.gpsimd]
    _dma_rr = [0]

    def dma_rr(out, in_):
        eng = _dma_engines[_dma_rr[0] % len(_dma_engines)]
        _dma_rr[0] += 1
        eng.dma_start(out, in_)

    # psum -> sbuf cast/copy round-robin (DVE / Act only: both read PSUM)
    _cast_rr = [0]

    def cast(out, in_):
        if _cast_rr[0] % 2 == 0:
            nc.vector.tensor_copy(out, in_)
        else:
            nc.scalar.copy(out, in_)
        _cast_rr[0] += 1

    # ---- persistent SBUF tensors ----
    def T(name, shape, dt=fp):
        return pool.tile(shape, dt, name=name, tag=name)

    ebf = T("ebf", [B, NB * B], bf)
    etbf = T("etbf", [B, NB * B], bf)
    t1bf = T("t1bf", [B, NB * B], bf)
    t1tbf = T("t1tbf", [B, NB * B], bf)
    ltp = T("ltp", [B, NLT * B], bf)
    wbf = T("wbf", [B, NB * D], bf)        # W block (i,j) at i*D+j*B
    wtbf = T("wtbf", [B, NB * D], bf)      # W^T block (a,b) at a*D+b*B
    vbuf = T("vbuf", [B, NB * CR], bf)
    xmu = T("xmu", [B, NB * WR], bf)       # [xq | mu] per k block
    tubf = T("tubf", [B, NB * WR], bf)     # [t | u] per i block
    t2x2 = T("t2x2", [B, NB * 2 * QS], bf)
    pbuf = T("pbuf", [B, NB * CR], bf)
    s2sb = T("s2sb", [B, NG * B], bf)      # per-group Neumann core S
    gbuf = T("gbuf", [B, NG * QS], bf)
    rsd32 = T("rsd32", [B, NG])
    dmsb = T("dmsb", [B, NG])
    trln = T("trln", [B, 2 * NG])          # [tr2 cols | lnD cols]
    tusb = T("tusb", [C, WR])
    xmusb = T("xmusb", [C, WR])
    tnxn = T("tnxn", [C, 2 * QS])
    un_sb = T("un_sb", [C, 1])
    mun_sb = T("mun_sb", [C, 1])
    bias_sb = T("bias_sb", [C, 1])
    logits_sb = T("logits_sb", [C, QS])
    scr64 = T("scr64", [C, C])
    # consts
    ident_bf = T("ident_bf", [B, B], bf)
    twoi_bf = T("twoi_bf", [B, B], bf)
    clsid_bf = T("clsid_bf", [GC, B], bf)
    ident32 = T("ident32", [B, B])
    ones32 = T("ones32", [B, 1])
    onesr_bf = T("onesr_bf", [B, C], bf)
    clsidT32 = T("clsidT32", [B, GC])
    maskb = T("maskb", [B, NG * C])
    maskb_bf = T("maskb_bf", [B, NG * C], bf)
    blkmask_bf = T("blkmask_bf", [B, B], bf)

    # ---- input DMAs (W-phase inputs first; spread queues) ----
    dma(ident_bf[:], ins["ident_bf"][:])
    dma(ebf[:], ins["ebf"][:])
    nc.gpsimd.dma_start(etbf[:], ins["etbf"][:])
    dma(ltp[:], ins["ltp"][:])
    nc.gpsimd.dma_start(vbuf.rearrange("p (b n) -> p b n", b=NB),
                        ins["vb"].rearrange("(b p) n -> p b n", p=B))
    dma(xmu.rearrange("p (b w) -> p b w", w=WR)[:, :, 0:QS],
        ins["xqt"].rearrange("(b p) n -> p b n", p=B))
    dma(xmu.rearrange("p (b w) -> p b w", w=WR)[:, :, QS:],
        ins["mub"].rearrange("(b p) c -> p b c", p=B))
    for cname, ct in (("twoi_bf", twoi_bf), ("clsid_bf", clsid_bf),
                      ("ident32", ident32), ("ones32", ones32),
                      ("onesr_bf", onesr_bf), ("clsidT32", clsidT32),
                      ("maskb", maskb), ("maskb_bf", maskb_bf), ("blkmask_bf", blkmask_bf)):
        dma_rr(ct[:], ins[cname][:])

    # zero the strictly-upper W blocks (read as zeros in fwd substitution)
    for k in range(NB - 1):
        nc.gpsimd.memset(wbf[:, k * D + (k + 1) * B: (k + 1) * D], 0.0)

    # T1 = I + E (broadcast identity across the 5 blocks)
    ib = ident_bf[:, None, :].broadcast_to([B, NB, B])
    nc.vector.tensor_add(t1bf.rearrange("p (b n) -> p b n", b=NB),
                         ebf.rearrange("p (b n) -> p b n", b=NB), ib)
    nc.vector.tensor_add(t1tbf.rearrange("p (b n) -> p b n", b=NB),
                         etbf.rearrange("p (b n) -> p b n", b=NB), ib)

    def _gate(n):
        if STOP_AFTER <= n:
            nc.vector.memset(logits_sb[:], 0.0)
            dma(ins["out"][:], logits_sb[:])
            for p in (psC, psN, psM, psA, spool, pool):
                p.release()
            return True
        return False

    eb = lambda b: ebf[:, b * B:(b + 1) * B]
    etb = lambda b: etbf[:, b * B:(b + 1) * B]
    t1b = lambda b: t1bf[:, b * B:(b + 1) * B]
    t1tb = lambda b: t1tbf[:, b * B:(b + 1) * B]
    w_blk = lambda i, j: wbf[:, i * D + j * B: i * D + (j + 1) * B]
    wt_blk = lambda a, b: wtbf[:, a * D + b * B: a * D + (b + 1) * B]
    _ltidx = {ki: j for j, ki in enumerate(_lt_pairs())}
    lt_blk = lambda k, i: ltp[:, _ltidx[(k, i)] * B:(_ltidx[(k, i)] + 1) * B]

    # =========== phase W-diag: W_bb = (I+E)(I+E^2), deg-3 Neumann ===========
    for b in range(NB):
        e2tps = psN.tile([B, B], fp, name=f"e2tps{b}", tag="nks")
        mm(e2tps[:], eb(b), etb(b), start=True, stop=True)
        e2tbf = spool.tile([B, B], bf, name=f"e2tbf{b}", tag="e2tbf", bufs=3)
        cast(e2tbf[:], e2tps[:])
        wps = psN.tile([B, B], fp, name=f"wps{b}", tag="nks")
        mm(wps[:], e2tbf[:], t1b(b), start=True, stop=False)
        mm(wps[:], ident_bf[:], t1b(b), start=False, stop=True)
        cast(w_blk(b, b), wps[:])
        wtps = psN.tile([B, B], fp, name=f"wtps{b}", tag="nks")
        mm(wtps[:], t1b(b), e2tbf[:], start=True, stop=False)
        mm(wtps[:], ident_bf[:], t1tb(b), start=False, stop=True)
        cast(wt_blk(b, b), wtps[:])

    # preload the Sqrt activation table off the critical path
    akscr = spool.tile([1, 1], fp, name="akscr", tag="akscr", bufs=1)
    nc.scalar.activation(akscr[:], ones32[0:1, 0:1], AF.Sqrt)

    if _gate(1):
        return
    # =========== phase W-offdiag: row-batched forward substitution ===========
    for i in range(1, NB):
        accps = psA.tile([B, i * B], fp, name=f"acc{i}", tag="bigA")
        for k in range(i):
            mm(accps[:], lt_blk(k, i), wbf[:, k * D: k * D + i * B],
               start=(k == 0), stop=(k == i - 1))
        tij = spool.tile([B, i * B], bf, name=f"tij{i}", tag="tij", bufs=3)
        cast(tij[:], accps[:])
        wps2 = psA.tile([B, i * B], fp, name=f"wo{i}", tag="bigA")
        mm(wps2[:], wt_blk(i, i), tij[:], start=True, stop=True)
        nc.vector.tensor_scalar(out=wbf[:, i * D: i * D + i * B], in0=wps2[:],
                                scalar1=-1.0, scalar2=None, op0=OP.mult)
        for j in range(i):
            trps = psN.tile([B, B], bf, name=f"tr{i}{j}", tag="nks")
            nc.tensor.transpose(trps[:], w_blk(i, j), ident_bf[:])
            cast(wt_blk(j, i), trps[:])

    if _gate(2):
        return
    # =========== P = W @ V (bf16) ===========
    for i in range(NB):
        for ch in range(2):
            pps = psA.tile([B, CR // 2], fp, name=f"p{i}{ch}", tag="bigA")
            for k in range(i + 1):
                mm(pps[:], wt_blk(k, i),
                   vbuf[:, k * CR + ch * (CR // 2): k * CR + (ch + 1) * (CR // 2)],
                   start=(k == 0), stop=(k == i))
            cast(pbuf[:, i * CR + ch * (CR // 2): i * CR + (ch + 1) * (CR // 2)],
                 pps[:])

    if _gate(3):
        return
    # =========== t = W xq, u = W mu (fused rhs = [xq | mu]) ===========
    for i in range(NB):
        tups = psA.tile([B, WR], fp, name=f"tu{i}", tag="bigA")
        for k in range(i + 1):
            mm(tups[:], wt_blk(k, i), xmu[:, k * WR:(k + 1) * WR],
               start=(k == 0), stop=(k == i))
        nc.vector.tensor_copy(tubf[:, i * WR:(i + 1) * WR], tups[:])
        nc.scalar.square(t2x2[:, i * 2 * QS: i * 2 * QS + QS], tups[:, 0:QS])
        nc.gpsimd.tensor_mul(t2x2[:, i * 2 * QS + QS:(i + 1) * 2 * QS],
                             xmu[:, i * WR: i * WR + QS],
                             xmu[:, i * WR: i * WR + QS])

    # =========== tu = u^T [t|u], xmu = mu^T [xq|mu] ===========
    ptu = psA.tile([C, WR], fp, name="ptu", tag="bigA")
    for k in range(NB):
        mm(ptu[:], tubf[:, k * WR + QS:(k + 1) * WR],
           tubf[:, k * WR:(k + 1) * WR], start=(k == 0), stop=(k == NB - 1))
    nc.scalar.copy(tusb[:], ptu[:])
    pxmu = psA.tile([C, WR], fp, name="pxmu", tag="bigA")
    for k in range(NB):
        mm(pxmu[:], xmu[:, k * WR + QS:(k + 1) * WR],
           xmu[:, k * WR:(k + 1) * WR], start=(k == 0), stop=(k == NB - 1))
    nc.scalar.copy(xmusb[:], pxmu[:])
    nc.vector.tensor_mul(scr64[:], tusb[:, QS:], ident32[0:C, 0:C])
    nc.vector.tensor_reduce(un_sb[:], scr64[:], AX.X, OP.add)
    nc.vector.tensor_mul(scr64[:], xmusb[:, QS:], ident32[0:C, 0:C])
    nc.vector.tensor_reduce(mun_sb[:], scr64[:], AX.X, OP.add)

    # =========== tn | xn row sums (replicated over classes) ===========
    ptn = psA.tile([C, 2 * QS], fp, name="ptn", tag="bigA")
    for bk in range(NB):
        mm(ptn[:], onesr_bf[:], t2x2[:, bk * 2 * QS:(bk + 1) * 2 * QS],
           start=(bk == 0), stop=(bk == NB - 1))
    nc.scalar.copy(tnxn[:], ptn[:])

    if _gate(4):
        return
    # =========== M_g = P_g^T P_g  (two [B,512] psum tiles, 4 groups each) ===========
    mts = [psM.tile([B, 4 * B], fp, name=f"mts{h}", tag="psM") for h in range(2)]
    mreg = lambda g: mts[g // 4][:, (g % 4) * B:(g % 4 + 1) * B]
    for g in range(NG):
        pslc = lambda k: pbuf[:, k * CR + g * B: k * CR + (g + 1) * B]
        for k in range(NB):
            mm(mreg(g), pslc(k), pslc(k), start=(k == 0), stop=False)
        mm(mreg(g), ident_bf[:], ident_bf[:], start=False, stop=True)  # M = I + P^T P

    # =========== NK feeds: diag, rsd, masked col-scale, J, I-J ===========
    mi_l, jbf_l, imj_l, msb_l = [], [], [], []
    for g in range(NG):
        msb = spool.tile([B, B], fp, name=f"msb{g}", tag="msb", bufs=NG)
        nc.scalar.copy(msb[:], mreg(g))
        msb_l.append(msb)
    for g in range(NG):
        mi = spool.tile([B, B], fp, name=f"mi{g}", tag="mi", bufs=NG)
        nc.gpsimd.tensor_mul(mi[:], msb_l[g][:], ident32[:])
        mi_l.append(mi)
    dm_l = []
    for g in range(NG):
        dmps = psN.tile([B, 1], fp, name=f"dm{g}", tag="nks")
        mm(dmps[:], mi_l[g][:], ones32[:], start=True, stop=True)
        dm_l.append(dmps)
    rsdbf_l = []
    for g in range(NG):
        nc.vector.tensor_copy(dmsb[:, g:g + 1], dm_l[g][:])
        rcp = spool.tile([B, 1], fp, name=f"rcp{g}", tag="rcp", bufs=NG)
        nc.vector.reciprocal(rcp[:], dm_l[g][:])
        rsdbf = spool.tile([B, 1], bf, name=f"rsdb{g}", tag="rsdb", bufs=NG)
        nc.scalar.activation(rsdbf[:], rcp[:], AF.Sqrt)
        rsdbf_l.append(rsdbf)
    half_l = []
    for g in range(NG):
        nc.vector.tensor_copy(rsd32[:, g:g + 1], rsdbf_l[g][:])
        half = spool.tile([B, B], bf, name=f"half{g}", tag="half", bufs=NG)
        nc.vector.scalar_tensor_tensor(
            out=half[:], in0=msb_l[g][:], scalar=rsd32[:, g:g + 1],
            in1=blkmask_bf[:], op0=OP.mult, op1=OP.mult)
        half_l.append(half)
    for g in range(NG):
        htp = psN.tile([B, B], bf, name=f"htp{g}", tag="nks")
        nc.tensor.transpose(htp[:], half_l[g][:], ident_bf[:])
        jraw = spool.tile([B, B], bf, name=f"jraw{g}", tag="jraw", bufs=NG)
        nc.vector.tensor_scalar(out=jraw[:], in0=htp[:],
                                scalar1=rsd32[:, g:g + 1], scalar2=None,
                                op0=OP.mult)
        jbf = spool.tile([B, B], bf, name=f"jbf{g}", tag="jbf", bufs=NG)
        nc.vector.tensor_sub(jbf[:], jraw[:], ident_bf[:])
        jbf_l.append(jbf)
        imj = spool.tile([B, B], bf, name=f"imj{g}", tag="imj", bufs=NG)
        nc.gpsimd.tensor_sub(imj[:], twoi_bf[:], jraw[:])
        imj_l.append(imj)

    if _gate(5):
        return
    # =========== g = P_g^T [t|u] - b, scaled by rsd ===========
    for g in range(NG):
        pg = psA.tile([B, WR], fp, name=f"pg{g}", tag="bigA")
        for k in range(NB):
            mm(pg[:], pbuf[:, k * CR + g * B: k * CR + (g + 1) * B],
               tubf[:, k * WR:(k + 1) * WR], start=(k == 0), stop=(k == NB - 1))
        bscr = spool.tile([B, C], fp, name=f"bscr{g}", tag="bscr", bufs=4)
        nc.vector.tensor_mul(bscr[:], pg[:, QS:], maskb[:, g * C:(g + 1) * C])
        bg = spool.tile([B, 1], fp, name=f"bg{g}", tag="bg", bufs=4)
        nc.vector.tensor_reduce(bg[:], bscr[:], AX.X, OP.add)
        nc.vector.tensor_scalar(out=gbuf[:, g * QS:(g + 1) * QS],
                                in0=pg[:, 0:QS], scalar1=bg[:],
                                scalar2=rsd32[:, g:g + 1],
                                op0=OP.subtract, op1=OP.mult)

    if _gate(6):
        return
    # =========== NK matmuls: S_g = (I + J^2)(I - J) ===========
    for g in range(NG):
        j2ps = psN.tile([B, B], fp, name=f"j2{g}", tag="nks")
        mm(j2ps[:], jbf_l[g][:], jbf_l[g][:], start=True, stop=True)
        j2bf = spool.tile([B, B], bf, name=f"j2b{g}", tag="j2b", bufs=NG)
        cast(j2bf[:], j2ps[:])
        s1ps = psN.tile([B, B], fp, name=f"s1{g}", tag="nks")
        mm(s1ps[:], j2bf[:], imj_l[g][:], start=True, stop=False)
        mm(s1ps[:], ident_bf[:], imj_l[g][:], start=False, stop=True)
        cast(s2sb[:, g * B:(g + 1) * B], s1ps[:])
        sq = spool.tile([B, B], fp, name=f"sq{g}", tag="sq", bufs=4)
        nc.scalar.square(sq[:], jbf_l[g][:])
        t2g = psN.tile([B, 1], fp, name=f"t2g{g}", tag="nks")
        mm(t2g[:], sq[:], ones32[:], start=True, stop=True)
        nc.vector.tensor_copy(trln[:, g:g + 1], t2g[:])

    # lnD after all Rsqrt ops (one act-table switch)
    nc.scalar.activation(trln[:, NG:], dmsb[:], AF.Ln)

    if _gate(7):
        return
    # =========== Kh, corr (two accumulators), class-summed bias ===========
    corrA = psC.tile([C, QS], fp, name="corrA", tag="corrA")
    corrB = psM.tile([C, QS], fp, name="corrB", tag="psM")
    for g in range(NG):
        hps = psA.tile([B, QS], fp, name=f"h{g}", tag="bigA")
        mm(hps[:], s2sb[:, g * B:(g + 1) * B], gbuf[:, g * QS:(g + 1) * QS],
           start=True, stop=True)
        prod = spool.tile([B, QS], bf, name=f"prod{g}", tag="prod", bufs=NG)
        if g % 2 == 0:
            nc.vector.tensor_mul(prod[:], hps[:], gbuf[:, g * QS:(g + 1) * QS])
        else:
            hsb = spool.tile([B, QS], fp, name=f"hsb{g}", tag="hsb", bufs=4)
            nc.scalar.copy(hsb[:], hps[:])
            nc.gpsimd.tensor_mul(prod[:], hsb[:], gbuf[:, g * QS:(g + 1) * QS])
        tgt = corrA if g % 2 == 0 else corrB
        mm(tgt[:], maskb_bf[:, g * C:(g + 1) * C], prod[:],
           start=(g < 2), stop=(g >= NG - 2))

    clsps = psN.tile([C, 2], fp, name="clsps", tag="nks")
    trv = trln.rearrange("p (two g) -> p g two", g=NG)
    for g in range(NG):
        mm(clsps[:], maskb[:, g * C:(g + 1) * C], trv[:, g, :],
           start=(g == 0), stop=(g == NG - 1))
    nc.vector.tensor_scalar(out=bias_sb[:], in0=clsps[:, 1:2], scalar1=-0.5,
                            scalar2=sc["BC0"], op0=OP.mult, op1=OP.add)
    nc.vector.scalar_tensor_tensor(out=bias_sb[:], in0=clsps[:, 0:1],
                                   scalar=0.25, in1=bias_sb[:],
                                   op0=OP.mult, op1=OP.add)

    if _gate(8):
        return
    # =========== assemble logits ===========
    wda = spool.tile([C, QS], fp, name="wda", tag="wda", bufs=1)
    d2a = spool.tile([C, QS], fp, name="d2a", tag="d2a", bufs=1)
    acc = spool.tile([C, QS], fp, name="acc", tag="acc", bufs=1)
    # wd2 = tn - 2*tu + un
    nc.vector.scalar_tensor_tensor(out=wda[:], in0=tusb[:, 0:QS], scalar=-2.0,
                                   in1=tnxn[:, 0:QS], op0=OP.mult, op1=OP.add)
    nc.vector.tensor_scalar(out=wda[:], in0=wda[:], scalar1=un_sb[:],
                            scalar2=None, op0=OP.add)
    # d2 + mun + common/REG
    nc.vector.scalar_tensor_tensor(out=d2a[:], in0=xmusb[:, 0:QS], scalar=-2.0,
                                   in1=tnxn[:, QS:], op0=OP.mult, op1=OP.add)
    nc.vector.tensor_scalar(out=d2a[:], in0=d2a[:], scalar1=mun_sb[:],
                            scalar2=sc["common"] / REG, op0=OP.add, op1=OP.add)
    # acc = ca*(wd2 - corrA - corrB) + REG*d2' = maha + common
    nc.vector.tensor_sub(acc[:], wda[:], corrA[:])
    nc.vector.tensor_sub(acc[:], acc[:], corrB[:])
    nc.vector.tensor_scalar(out=acc[:], in0=acc[:], scalar1=sc["ca"],
                            scalar2=None, op0=OP.mult)
    nc.vector.scalar_tensor_tensor(out=acc[:], in0=d2a[:], scalar=REG,
                                   in1=acc[:], op0=OP.mult, op1=OP.add)
    if DEBUG_DUMP:
        corrs = spool.tile([C, QS], fp, name="corrs", tag="corrs", bufs=1)
        nc.vector.tensor_add(corrs[:], corrA[:], corrB[:])
        for nm, t in (("d_rsd32", rsd32), ("d_dmsb", dmsb), ("d_trln", trln),
                      ("d_bias", bias_sb), ("d_s2", s2sb), ("d_gbuf", gbuf),
                      ("d_tusb", tusb), ("d_tnxn", tnxn), ("d_wbf", wbf),
                      ("d_pbuf", pbuf), ("d_tubf", tubf), ("d_corr", corrs)):
            dma(ins[nm][:], t[:])
    nc.scalar.activation(acc[:], acc[:], AF.Ln)
    nc.vector.tensor_scalar(out=logits_sb[:], in0=acc[:], scalar1=-sc["CC"],
                            scalar2=bias_sb[:], op0=OP.mult, op1=OP.add)
    dma(ins["out"][:], logits_sb[:])

    for p in (psC, psN, psM, psA, spool, pool):
        p.release()


def build_program(sc):
    import concourse.mybir as mybir
    import concourse.tile as tile
    from concourse import bacc

    nc = bacc.Bacc("TRN2", target_bir_lowering=False, debug=False,
                   num_devices=NCORES)
    fp = mybir.dt.float32
    bf = mybir.dt.bfloat16
    shapes = dict(
        ebf=([B, NB * B], bf), etbf=([B, NB * B], bf),
        ltp=([B, NLT * B], bf), vb=([D, C * R], bf), mub=([D, C], bf),
        xqt=([D, QS], bf),
        ident_bf=([B, B], bf), twoi_bf=([B, B], bf), clsid_bf=([GC, B], bf),
        ident32=([B, B], fp), ones32=([B, 1], fp), onesr_bf=([B, C], bf),
        clsidT32=([B, GC], fp), maskb=([B, NG * C], fp),
        maskb_bf=([B, NG * C], bf), blkmask_bf=([B, B], bf),
    )
    ins = {name: nc.dram_tensor(name, shp, dt, kind="ExternalInput").ap()
           for name, (shp, dt) in shapes.items()}
    ins["out"] = nc.dram_tensor("out", [C, QS], fp, kind="ExternalOutput").ap()
    if DEBUG_DUMP:
        for nm, shp, dt in (("d_rsd32", [B, NG], fp), ("d_dmsb", [B, NG], fp),
                            ("d_trln", [B, 2 * NG], fp), ("d_bias", [C, 1], fp),
                            ("d_s2", [B, NG * B], bf), ("d_gbuf", [B, NG * QS], bf),
                            ("d_tusb", [C, WR], fp), ("d_tnxn", [C, 2 * QS], fp),
                            ("d_wbf", [B, NB * D], bf), ("d_pbuf", [B, NB * CR], bf),
                            ("d_tubf", [B, NB * WR], bf), ("d_corr", [C, QS], fp)):
            ins[nm] = nc.dram_tensor(nm, shp, dt, kind="ExternalOutput").ap()
    with tile.TileContext(nc) as tc:
        _emit(nc, tc, ins, sc)
    nc.compile()
    return nc


_BUILD_CACHE = {}


def kernel(**inputs) -> np.ndarray:
    from concourse import bass_utils

    shared, xqts, sc = _host_prep(inputs)
    key = tuple(sorted(sc.items()))
    if key not in _BUILD_CACHE:
        _BUILD_CACHE[key] = build_program(sc)
    nc = _BUILD_CACHE[key]

    in_maps = []
    for c in range(NCORES):
        im = {k: v for k, v in shared.items()}
        im["xqt"] = xqts[c]
        in_maps.append(im)
    res = bass_utils.run_bass_kernel_spmd(nc, in_maps, core_ids=list(range(NCORES)))
    logits = np.concatenate([r["out"].T for r in res.results], axis=0)
    return logits.astype(np.float32)


if __name__ == "__main__":
    rng = np.random.default_rng(0)
    demo = dict(
        X_support=rng.standard_normal((C * S, D), dtype=np.float32),
        y=np.repeat(np.arange(C, dtype=np.int64), S),
        X_query=rng.standard_normal((Q, D), dtype=np.float32),
        m=0.01 * rng.standard_normal((1, D)).astype(np.float32),
        kappa=np.float32(0.1), nu=np.float32(D),
        triu_diag=np.ones(D, dtype=np.float32),
        triu_lower=(np.eye(D) + 0.01 * rng.standard_normal((D, D))).astype(np.float32),
    )
    out = kernel(**demo)
    print(out.shape, out.dtype, np.abs(out).max())
